# revision 28
# baseline (speedup 1.0000x reference)
"""Trainium2 Bass kernel for nn_FeatureConverge_82145544503995.

Data-parallel over the batch dim: 8 batches per NeuronCore x 8 cores.
Per batch, on-device:
  - conv1 (weight-normed 3-tap 1D conv over seq) + tanh(tanh(.)) -> x_site
  - hardware dma_gather of 8320 rows (512 B each) of x[b], index order
    chosen so the gather lands directly in a slot-aligned parity layout:
    partition g holds E[m]=ctx[g,2m,:] at slot m, partition 64+g holds
    O[m]=ctx[g,2m+1,:] at slot m+1.  Gathers round-robin over the 4 SWDGE
    queues so descriptor generation runs on all four Q7 core pairs.
  - conv2 (3x3 grouped conv over the gathered [G,K,C] block) as 6 dense
    K=128 accumulating PE matmuls per 512-element output chunk (weight
    blocks A/B per c-tap), plus 8 N=64 matmuls per batch that recompute
    the two c-edge output columns exactly (the main pass reads one
    garbage element across slot boundaries there).
All matmuls run in float32r (full PE rate at N>=256).
"""
import numpy as np

B, S, C, G, K = 64, 2048, 128, 64, 128
NCORES = 8
BPC = B // NCORES        # batches per core
NSLOT = K // 2           # 64 (even,odd) k-pair data slots
NSL = NSLOT + 1          # 65 slots incl. boundary halves
IDXN = NSL * 128         # 8320 gather indices per batch
_CACHE = {}


def _build_nc():
    from contextlib import ExitStack

    import concourse.bacc as bacc
    import concourse.mybir as mybir
    import concourse.tile as tile
    from concourse.tile_rust import add_dep_helper

    f32 = mybir.dt.float32
    f32r = mybir.dt.float32r
    i16 = mybir.dt.int16
    TANH = mybir.ActivationFunctionType.Tanh
    IDENT = mybir.ActivationFunctionType.Identity

    import concourse.hw_specs as hw_specs
    _old_ns = hw_specs.TRN2Spec.SWDGE_NS_PER_DESCRIPTOR
    if int(__import__('os').environ.get('COSTPATCH', '1')):
        hw_specs.TRN2Spec.SWDGE_NS_PER_DESCRIPTOR = 7.8  # measured Q7 gen rate
    nc = bacc.Bacc("TRN2", target_bir_lowering=False, debug=False,
                   enable_asserts=True, num_devices=NCORES,
                   num_swdge_queues=4)
    x_d = nc.dram_tensor("x", [BPC, S + 1, C], f32, kind="ExternalInput")
    idx_d = nc.dram_tensor("idx16", [BPC, 128, IDXN // 16], i16,
                           kind="ExternalInput")
    w1_d = nc.dram_tensor("w1pack", [C, 3 * C], f32, kind="ExternalInput")
    w2_d = nc.dram_tensor("w2pack", [128, 768], f32, kind="ExternalInput")
    b1_d = nc.dram_tensor("b1col", [C, 1], f32, kind="ExternalInput")
    b2_d = nc.dram_tensor("b2col", [128, 1], f32, kind="ExternalInput")
    id_d = nc.dram_tensor("ident", [128, 128], f32, kind="ExternalInput")
    xbag_d = nc.dram_tensor("xbag", [BPC, G, K, C], f32, kind="ExternalOutput")
    xsite_d = nc.dram_tensor("xsite", [BPC, C, S], f32, kind="ExternalOutput")

    with tile.TileContext(nc) as tc, ExitStack() as es:
        const = es.enter_context(tc.tile_pool(name="const", bufs=1))
        pidx = es.enter_context(tc.tile_pool(name="pidx", bufs=8))
        praw = es.enter_context(tc.tile_pool(name="praw", bufs=4))
        pxs = es.enter_context(tc.tile_pool(name="pxs", bufs=2))
        pxp = es.enter_context(tc.tile_pool(name="pxp", bufs=2))
        psite = es.enter_context(tc.tile_pool(name="psite", bufs=3))
        pbag = es.enter_context(tc.tile_pool(name="pbag", bufs=4))
        psT = es.enter_context(tc.tile_pool(name="psT", bufs=2, space="PSUM"))
        psY = es.enter_context(tc.tile_pool(name="psY", bufs=2, space="PSUM"))
        psA = es.enter_context(tc.tile_pool(name="psA", bufs=3, space="PSUM"))
        psE = es.enter_context(tc.tile_pool(name="psE", bufs=1, space="PSUM"))

        w1s_t = const.tile([C, 3 * C], f32)
        nc.sync.dma_start(w1s_t[:], w1_d.ap())
        w2s_t = const.tile([128, 768], f32)
        nc.sync.dma_start(w2s_t[:], w2_d.ap())
        w1_t = const.tile([C, 3 * C], f32r)
        nc.vector.tensor_copy(w1_t[:], w1s_t[:])
        w2_t = const.tile([128, 768], f32r)
        nc.vector.tensor_copy(w2_t[:], w2s_t[:])
        id_t = const.tile([128, 128], f32r)
        nc.sync.dma_start(id_t[:], id_d.ap().bitcast(f32r))
        b1_t = const.tile([C, 1], f32)
        nc.sync.dma_start(b1_t[:], b1_d.ap())
        b2_t = const.tile([128, 1], f32)
        nc.sync.dma_start(b2_t[:], b2_d.ap())
        z_t = const.tile([128, C], f32)
        nc.vector.memset(z_t[:], 0.0)

        def wA(tb):
            return w2_t[:, tb * 128:(tb + 1) * 128]

        def wB(tb):
            return w2_t[:, 384 + tb * 128:384 + (tb + 1) * 128]

        def edge_rhs(raw_t, off):
            # [128, 64(,1)]: one column per data slot, stride 128 elems
            v = raw_t[:, 2:2 + IDXN].rearrange("p (j c) -> p j c", c=C)
            return v[:, off // C:off // C + NSLOT, off % C:off % C + 1]

        # PE warmup: ~5us of dense matmuls so HAM reaches 8/8 before the
        # first real conv work lands.
        if int(__import__('os').environ.get('WARMUP', '1')):
            pw = psY.tile([128, 512], f32, tag="py")
            for i in range(24):
                nc.tensor.matmul(pw[:], w2_t[:, 0:128],
                                 w2_t[:, 128:640],
                                 start=(i == 0), stop=(i == 23))

        def gather_part(b):
            # ---- slot-aligned parity gather ----
            idx_t = pidx.tile([128, IDXN // 16], i16)
            nc.sync.dma_start(idx_t[:], idx_d.ap()[b])
            raw_t = praw.tile([128, IDXN + 4], f32r)
            # pre-trigger zeroing of the 2-element prefix/suffix (the
            # boundary half-slots gather the real zero row x[b, S, :]).
            nc.vector.tensor_copy(raw_t[:, 0:2], z_t[:, 0:2])
            nc.vector.tensor_copy(raw_t[:, 2 + IDXN:4 + IDXN], z_t[:, 0:2])
            nc.gpsimd.dma_gather(
                out_ap=raw_t[:, 2:2 + IDXN].rearrange("p (j c) -> p j c", c=C),
                in_ap=x_d.ap()[b].bitcast(f32r),
                idxs_ap=idx_t[:],
                num_idxs=IDXN,
                num_idxs_reg=IDXN,
                elem_size=C,
                single_packet=False,
                queue_num=b % 4,
            )
            return raw_t

        def conv1_part(b):
            # ---- conv1: PE-transpose x[b] -> [C, S+2], 3-tap conv, 2x tanh ----
            xs_t = pxs.tile([128, S], f32r)
            nc.sync.dma_start(
                xs_t[:].rearrange("p (t c) -> p t c", c=C),
                x_d.ap()[b][0:S].rearrange("(t p) c -> p t c", p=128)
                    .bitcast(f32r),
            )
            xp_t = pxp.tile([128, S + 2], f32r)
            nc.vector.tensor_copy(xp_t[:, 0:1], z_t[:, 0:1])
            nc.vector.tensor_copy(xp_t[:, S + 1:S + 2], z_t[:, 0:1])
            for q in range(S // 512):
                pt = psT.tile([128, 512], f32r)
                for u in range(4):
                    nc.tensor.matmul(
                        pt[:, u * 128:(u + 1) * 128],
                        xs_t[:, (q * 4 + u) * 128:(q * 4 + u + 1) * 128],
                        id_t[:], is_transpose=True,
                        start=(u == 0), stop=(u == 3),
                    )
                dst = xp_t[:, 1 + q * 512:1 + (q + 1) * 512]
                nc.scalar.copy(dst, pt[:])
            for n0 in range(0, S, 512):
                py = psY.tile([128, 512], f32)
                for t in range(3):
                    last_mm = nc.tensor.matmul(
                        py[:],
                        w1_t[:, t * C:(t + 1) * C],
                        xp_t[:, n0 + t:n0 + t + 512],
                        start=(t == 0), stop=(t == 2),
                    )
                s1 = psite.tile([128, 512], f32)
                nc.scalar.activation(s1[:], py[:], TANH, bias=b1_t[:])
                s2 = psite.tile([128, 512], f32)
                nc.scalar.activation(s2[:], s1[:], TANH)
                nc.sync.dma_start(xsite_d.ap()[b][:, n0:n0 + 512], s2[:])
            return last_mm


        def batch_tail(b, raw_t, conv1_fence):
            # ---- conv2 edge columns (c=0 -> pe[:,0:64], c=127 -> pe[:,64:128]) ----
            pe = psE.tile([128, 128], f32)
            edge_seq = [
                (0, 0, 1, 0), (0, 0, 2, 1), (0, 1, 1, 128), (0, 1, 2, 129),
                (1, 0, 0, 126), (1, 0, 1, 127), (1, 1, 0, 254), (1, 1, 1, 255),
            ]
            for i, (eo, blk, tb, off) in enumerate(edge_seq):
                emm = nc.tensor.matmul(
                    pe[:, eo * 64:eo * 64 + 64],
                    wA(tb) if blk == 0 else wB(tb),
                    edge_rhs(raw_t, off),
                    start=(i == 0), stop=(i == 7),
                )
                if i == 0 and conv1_fence is not None:
                    # ordering-only edge: keep every conv2 tail behind the
                    # gather-independent conv1 work on the PE stream
                    add_dep_helper(emm.ins, conv1_fence.ins, sync=False,
                                   reason="conv1 fills gather latency")

            # ---- conv2 main: 16 chunk-pairs of 8 k-rows x 128 c ----
            for ci, m0 in enumerate(range(0, NSLOT, 4)):
                ps = psA.tile([128, 512], f32)
                seq = [(0, 1), (0, 0), (0, 2), (1, 1), (1, 0), (1, 2)]
                for i, (blk, tb) in enumerate(seq):
                    base = 2 + (m0 + blk) * 128 + tb - 1
                    nc.tensor.matmul(
                        ps[:],
                        wA(tb) if blk == 0 else wB(tb),
                        raw_t[:, base:base + 512],
                        start=(i == 0), stop=(i == 5),
                    )
                bag = pbag.tile([128, 512], f32)
                bagv = bag[:].rearrange("p (j c) -> p j c", c=C)
                pev = pe[:].rearrange("p (e m) -> p e m", m=64)
                e0 = pev[:, 0, m0:m0 + 4].rearrange("p (m one) -> p m one", one=1)
                e1 = pev[:, 1, m0:m0 + 4].rearrange("p (m one) -> p m one", one=1)
                if ci % 2 == 0:
                    nc.vector.tensor_scalar_add(bag[:], ps[:], b2_t[:])
                    nc.scalar.activation(bagv[:, :, 0:1], e0, IDENT, bias=b2_t[:])
                    nc.vector.tensor_scalar_add(bagv[:, :, C - 1:C], e1, b2_t[:])
                else:
                    nc.scalar.activation(bag[:], ps[:], IDENT, bias=b2_t[:])
                    nc.vector.tensor_scalar_add(bagv[:, :, 0:1], e0, b2_t[:])
                    nc.scalar.activation(bagv[:, :, C - 1:C], e1, IDENT, bias=b2_t[:])
                dst = xbag_d.ap()[b].rearrange("g (m two) c -> two g m c", two=2)
                bv = bag[:].rearrange("p (j c) -> p j c", c=C)
                nc.sync.dma_start(dst[0][:, m0:m0 + 4, :], bv[0:64])
                nc.sync.dma_start(dst[1][:, m0:m0 + 4, :], bv[64:128])

        # software pipeline: first 4 gathers up front (praw bufs), then all
        # gather-independent conv1 work to keep the PE dense while gathers
        # stream, then conv2 tails interleaved with the remaining gathers.
        raws = {}
        for b in range(4):
            raws[b] = gather_part(b)
        fence = None
        for b in range(BPC):
            fence = conv1_part(b)
        for b in range(BPC):
            batch_tail(b, raws.pop(b), fence)
            if b + 4 < BPC:
                raws[b + 4] = gather_part(b + 4)

    hw_specs.TRN2Spec.SWDGE_NS_PER_DESCRIPTOR = _old_ns
    nc.compile()
    return nc


def get_nc():
    if "nc" not in _CACHE:
        _CACHE["nc"] = _build_nc()
    return _CACHE["nc"]


def prep_shared(v1, g1, b1, W2, b2):
    """Host-side weight packing (tiny tensors)."""
    v1 = np.asarray(v1, np.float32)
    vnorm = np.sqrt((v1.astype(np.float64) ** 2).sum(axis=(1, 2), keepdims=True))
    w1 = (np.asarray(g1, np.float32)[:, None, None] * v1 / vnorm).astype(np.float32)
    w1pack = np.ascontiguousarray(w1.transpose(1, 2, 0).reshape(C, 3 * C))

    Wt = np.asarray(W2, np.float32).transpose(1, 0, 2, 3)  # [gi, go, a, tap]
    w2pack = np.zeros((128, 768), np.float32)
    for tb in range(3):
        # block A: rhs slot j -> (E[j], O[j-1])
        w2pack[0:64, tb * 128:tb * 128 + 64] = Wt[:, :, 1, tb]        # E->even a=1
        w2pack[0:64, tb * 128 + 64:tb * 128 + 128] = Wt[:, :, 0, tb]  # E->odd  a=0
        w2pack[64:128, tb * 128:tb * 128 + 64] = Wt[:, :, 0, tb]      # O->even a=0
        # block B: rhs slot j+1 -> (E[j+1], O[j])
        w2pack[0:64, 384 + tb * 128 + 64:384 + tb * 128 + 128] = Wt[:, :, 2, tb]   # E->odd a=2
        w2pack[64:128, 384 + tb * 128:384 + tb * 128 + 64] = Wt[:, :, 2, tb]       # O->even a=2
        w2pack[64:128, 384 + tb * 128 + 64:384 + tb * 128 + 128] = Wt[:, :, 1, tb] # O->odd a=1

    b1col = np.asarray(b1, np.float32).reshape(C, 1)
    b2col = np.concatenate([np.asarray(b2, np.float32)] * 2).reshape(128, 1)
    ident = np.eye(128, dtype=np.float32)
    return w1pack, w2pack, b1col, b2col, ident


def prep_idx(idx_local):
    """[BPC, G, K] int -> slot-aligned int16 index list: position
    j*128 + g = index of ctx row 2j (E[j]); position j*128 + 64 + g =
    index of ctx row 2j-1 (O[j-1]); boundary halves use index 0 (their
    gathered garbage is zeroed on-chip)."""
    a = np.asarray(idx_local)
    m = np.full((BPC, NSL, 128), S, np.int16)  # boundary -> zero row x[b, S]
    m[:, 0:NSLOT, 0:64] = a[:, :, 0::2].transpose(0, 2, 1)
    m[:, 1:NSL, 64:128] = a[:, :, 1::2].transpose(0, 2, 1)
    flat = m.reshape(BPC, IDXN)
    wrapped = flat.reshape(BPC, IDXN // 16, 16).transpose(0, 2, 1)
    return np.ascontiguousarray(np.tile(wrapped, (1, 8, 1)))


def make_in_maps(x, index_all, v1, g1, b1, W2, b2):
    x = np.asarray(x, np.float32)
    w1pack, w2pack, b1col, b2col, ident = prep_shared(v1, g1, b1, W2, b2)
    in_maps = []
    for c in range(NCORES):
        sl = slice(c * BPC, (c + 1) * BPC)
        xp = np.concatenate(
            [x[sl], np.zeros((BPC, 1, C), np.float32)], axis=1)
        in_maps.append({
            "x": np.ascontiguousarray(xp),
            "idx16": prep_idx(np.asarray(index_all)[sl]),
            "w1pack": w1pack,
            "w2pack": w2pack,
            "b1col": b1col,
            "b2col": b2col,
            "ident": ident,
        })
    return in_maps


# test harness hooks: set TRACE=True before calling kernel() to profile.
TRACE = False
LAST_RESULTS = {}


def kernel(x, index_all, v1, g1, b1, W2, b2):
    from concourse.bass_utils import run_bass_kernel_spmd

    x = np.asarray(x, np.float32)
    nc = get_nc()
    in_maps = make_in_maps(x, index_all, v1, g1, b1, W2, b2)
    res = run_bass_kernel_spmd(nc, in_maps, core_ids=list(range(NCORES)),
                               trace=TRACE)
    LAST_RESULTS["res"] = res
    xbag = np.concatenate([res.results[c]["xbag"] for c in range(NCORES)], axis=0)
    xsite = np.concatenate([res.results[c]["xsite"] for c in range(NCORES)], axis=0)
    asite = np.ascontiguousarray(x[:, 20:21, :])
    return xbag, asite, xsite


# revision 29
# speedup vs baseline: 1.1396x; 1.1396x over previous
"""Trainium2 Bass kernel for nn_FeatureConverge_82145544503995.

Data-parallel over the batch dim: 8 batches per NeuronCore x 8 cores.
Per batch, on-device:
  - conv1 (weight-normed 3-tap 1D conv over seq) + tanh(tanh(.)) -> x_site
  - hardware dma_gather of 8320 rows (512 B each) of x[b], index order
    chosen so the gather lands directly in a slot-aligned parity layout:
    partition g holds E[m]=ctx[g,2m,:] at slot m, partition 64+g holds
    O[m]=ctx[g,2m+1,:] at slot m+1.  Gathers round-robin over the 4 SWDGE
    queues so descriptor generation runs on all four Q7 core pairs.
  - conv2 (3x3 grouped conv over the gathered [G,K,C] block) as 6 dense
    K=128 accumulating PE matmuls per 512-element output chunk (weight
    blocks A/B per c-tap), plus 8 N=64 matmuls per batch that recompute
    the two c-edge output columns exactly (the main pass reads one
    garbage element across slot boundaries there).
All matmuls run in float32r (full PE rate at N>=256).
"""
import numpy as np

B, S, C, G, K = 64, 2048, 128, 64, 128
NCORES = 8
BPC = B // NCORES        # batches per core
NSLOT = K // 2           # 64 (even,odd) k-pair data slots
NSL = NSLOT + 1          # 65 slots incl. boundary halves
IDXN = NSL * 128         # 8320 gather indices per batch
_CACHE = {}


def _build_nc():
    from contextlib import ExitStack

    import concourse.bacc as bacc
    import concourse.mybir as mybir
    import concourse.tile as tile
    from concourse.tile_rust import add_dep_helper

    f32 = mybir.dt.float32
    f32r = mybir.dt.float32r
    i16 = mybir.dt.int16
    TANH = mybir.ActivationFunctionType.Tanh
    IDENT = mybir.ActivationFunctionType.Identity

    import concourse.hw_specs as hw_specs
    _old_ns = hw_specs.TRN2Spec.SWDGE_NS_PER_DESCRIPTOR
    if int(__import__('os').environ.get('COSTPATCH', '1')):
        hw_specs.TRN2Spec.SWDGE_NS_PER_DESCRIPTOR = 7.8  # measured Q7 gen rate
    nc = bacc.Bacc("TRN2", target_bir_lowering=False, debug=False,
                   enable_asserts=True, num_devices=NCORES,
                   num_swdge_queues=4)
    x_d = nc.dram_tensor("x", [BPC, S + 1, C], f32, kind="ExternalInput")
    idx_d = nc.dram_tensor("idx16", [BPC, 128, IDXN // 16], i16,
                           kind="ExternalInput")
    w1_d = nc.dram_tensor("w1pack", [C, 3 * C], f32, kind="ExternalInput")
    w2_d = nc.dram_tensor("w2pack", [128, 768], f32, kind="ExternalInput")
    b1_d = nc.dram_tensor("b1col", [C, 1], f32, kind="ExternalInput")
    b2_d = nc.dram_tensor("b2col", [128, 1], f32, kind="ExternalInput")
    id_d = nc.dram_tensor("ident", [128, 128], f32, kind="ExternalInput")
    xbag_d = nc.dram_tensor("xbag", [BPC, G, K, C], f32, kind="ExternalOutput")
    xsite_d = nc.dram_tensor("xsite", [BPC, C, S], f32, kind="ExternalOutput")

    with tile.TileContext(nc) as tc, ExitStack() as es:
        const = es.enter_context(tc.tile_pool(name="const", bufs=1))
        pidx = es.enter_context(tc.tile_pool(name="pidx", bufs=8))
        praw = es.enter_context(tc.tile_pool(name="praw", bufs=4))
        pxs = es.enter_context(tc.tile_pool(name="pxs", bufs=2))
        pxp = es.enter_context(tc.tile_pool(name="pxp", bufs=2))
        psite = es.enter_context(tc.tile_pool(name="psite", bufs=3))
        pbag = es.enter_context(tc.tile_pool(name="pbag", bufs=4))
        psT = es.enter_context(tc.tile_pool(name="psT", bufs=2, space="PSUM"))
        psY = es.enter_context(tc.tile_pool(name="psY", bufs=2, space="PSUM"))
        psA = es.enter_context(tc.tile_pool(name="psA", bufs=3, space="PSUM"))
        psE = es.enter_context(tc.tile_pool(name="psE", bufs=1, space="PSUM"))

        w1s_t = const.tile([C, 3 * C], f32)
        nc.sync.dma_start(w1s_t[:], w1_d.ap())
        w2s_t = const.tile([128, 768], f32)
        nc.sync.dma_start(w2s_t[:], w2_d.ap())
        w1_t = const.tile([C, 3 * C], f32r)
        nc.vector.tensor_copy(w1_t[:], w1s_t[:])
        w2_t = const.tile([128, 768], f32r)
        nc.vector.tensor_copy(w2_t[:], w2s_t[:])
        id_t = const.tile([128, 128], f32r)
        nc.sync.dma_start(id_t[:], id_d.ap().bitcast(f32r))
        b1_t = const.tile([C, 1], f32)
        nc.sync.dma_start(b1_t[:], b1_d.ap())
        b2_t = const.tile([128, 1], f32)
        nc.sync.dma_start(b2_t[:], b2_d.ap())
        z_t = const.tile([128, C], f32)
        nc.vector.memset(z_t[:], 0.0)

        def wA(tb):
            return w2_t[:, tb * 128:(tb + 1) * 128]

        def wB(tb):
            return w2_t[:, 384 + tb * 128:384 + (tb + 1) * 128]

        def edge_rhs(raw_t, off):
            # [128, 64(,1)]: one column per data slot, stride 128 elems
            v = raw_t[:, 2:2 + IDXN].rearrange("p (j c) -> p j c", c=C)
            return v[:, off // C:off // C + NSLOT, off % C:off % C + 1]

        # PE warmup: ~5us of dense matmuls so HAM reaches 8/8 before the
        # first real conv work lands.
        if int(__import__('os').environ.get('WARMUP', '1')):
            pw = psY.tile([128, 512], f32, tag="py")
            for i in range(24):
                nc.tensor.matmul(pw[:], w2_t[:, 0:128],
                                 w2_t[:, 128:640],
                                 start=(i == 0), stop=(i == 23))

        def gather_part(b):
            # ---- slot-aligned parity gather ----
            idx_t = pidx.tile([128, IDXN // 16], i16)
            nc.sync.dma_start(idx_t[:], idx_d.ap()[b])
            raw_t = praw.tile([128, IDXN + 4], f32r)
            # pre-trigger zeroing of the 2-element prefix/suffix (the
            # boundary half-slots gather the real zero row x[b, S, :]).
            nc.vector.tensor_copy(raw_t[:, 0:2], z_t[:, 0:2])
            nc.vector.tensor_copy(raw_t[:, 2 + IDXN:4 + IDXN], z_t[:, 0:2])
            nc.gpsimd.dma_gather(
                out_ap=raw_t[:, 2:2 + IDXN].rearrange("p (j c) -> p j c", c=C),
                in_ap=x_d.ap()[b].bitcast(f32r),
                idxs_ap=idx_t[:],
                num_idxs=IDXN,
                num_idxs_reg=IDXN,
                elem_size=C,
                single_packet=False,
                queue_num=b % 4,
            )
            return raw_t

        def conv1_part(b):
            # ---- conv1: PE-transpose x[b] -> [C, S+2], 3-tap conv, 2x tanh ----
            xs_t = pxs.tile([128, S], f32r)
            nc.sync.dma_start(
                xs_t[:].rearrange("p (t c) -> p t c", c=C),
                x_d.ap()[b][0:S].rearrange("(t p) c -> p t c", p=128)
                    .bitcast(f32r),
            )
            xp_t = pxp.tile([128, S + 2], f32r)
            nc.vector.tensor_copy(xp_t[:, 0:1], z_t[:, 0:1])
            nc.vector.tensor_copy(xp_t[:, S + 1:S + 2], z_t[:, 0:1])
            for q in range(S // 512):
                pt = psT.tile([128, 512], f32r)
                for u in range(4):
                    nc.tensor.matmul(
                        pt[:, u * 128:(u + 1) * 128],
                        xs_t[:, (q * 4 + u) * 128:(q * 4 + u + 1) * 128],
                        id_t[:], is_transpose=True,
                        start=(u == 0), stop=(u == 3),
                    )
                dst = xp_t[:, 1 + q * 512:1 + (q + 1) * 512]
                nc.scalar.copy(dst, pt[:])
            for n0 in range(0, S, 512):
                py = psY.tile([128, 512], f32)
                for t in range(3):
                    last_mm = nc.tensor.matmul(
                        py[:],
                        w1_t[:, t * C:(t + 1) * C],
                        xp_t[:, n0 + t:n0 + t + 512],
                        start=(t == 0), stop=(t == 2),
                    )
                s1 = psite.tile([128, 512], f32)
                nc.scalar.activation(s1[:], py[:], TANH, bias=b1_t[:])
                s2 = psite.tile([128, 512], f32)
                nc.scalar.activation(s2[:], s1[:], TANH)
                nc.sync.dma_start(xsite_d.ap()[b][:, n0:n0 + 512], s2[:])
            return last_mm


        def batch_tail(b, raw_t, conv1_fence):
            # ---- conv2 edge columns (c=0 -> pe[:,0:64], c=127 -> pe[:,64:128]) ----
            pe = psE.tile([128, 128], f32)
            edge_seq = [
                (0, 0, 1, 0), (0, 0, 2, 1), (0, 1, 1, 128), (0, 1, 2, 129),
                (1, 0, 0, 126), (1, 0, 1, 127), (1, 1, 0, 254), (1, 1, 1, 255),
            ]
            for i, (eo, blk, tb, off) in enumerate(edge_seq):
                emm = nc.tensor.matmul(
                    pe[:, eo * 64:eo * 64 + 64],
                    wA(tb) if blk == 0 else wB(tb),
                    edge_rhs(raw_t, off),
                    start=(i == 0), stop=(i == 7),
                )
                if i == 0 and conv1_fence is not None:
                    # ordering-only edge: keep every conv2 tail behind the
                    # gather-independent conv1 work on the PE stream
                    add_dep_helper(emm.ins, conv1_fence.ins, sync=False,
                                   reason="conv1 fills gather latency")

            # ---- conv2 main: 16 chunk-pairs of 8 k-rows x 128 c ----
            for ci, m0 in enumerate(range(0, NSLOT, 4)):
                ps = psA.tile([128, 512], f32)
                seq = [(0, 1), (0, 0), (0, 2), (1, 1), (1, 0), (1, 2)]
                for i, (blk, tb) in enumerate(seq):
                    base = 2 + (m0 + blk) * 128 + tb - 1
                    cmm = nc.tensor.matmul(
                        ps[:],
                        wA(tb) if blk == 0 else wB(tb),
                        raw_t[:, base:base + 512],
                        start=(i == 0), stop=(i == 5),
                    )
                    if i == 0 and conv1_fence is not None:
                        add_dep_helper(cmm.ins, conv1_fence.ins, sync=False,
                                       reason="conv1 fills gather latency")
                bag = pbag.tile([128, 512], f32)
                bagv = bag[:].rearrange("p (j c) -> p j c", c=C)
                pev = pe[:].rearrange("p (e m) -> p e m", m=64)
                e0 = pev[:, 0, m0:m0 + 4].rearrange("p (m one) -> p m one", one=1)
                e1 = pev[:, 1, m0:m0 + 4].rearrange("p (m one) -> p m one", one=1)
                if ci % 2 == 0:
                    nc.vector.tensor_scalar_add(bag[:], ps[:], b2_t[:])
                    nc.scalar.activation(bagv[:, :, 0:1], e0, IDENT, bias=b2_t[:])
                    nc.vector.tensor_scalar_add(bagv[:, :, C - 1:C], e1, b2_t[:])
                else:
                    nc.scalar.activation(bag[:], ps[:], IDENT, bias=b2_t[:])
                    nc.vector.tensor_scalar_add(bagv[:, :, 0:1], e0, b2_t[:])
                    nc.scalar.activation(bagv[:, :, C - 1:C], e1, IDENT, bias=b2_t[:])
                dst = xbag_d.ap()[b].rearrange("g (m two) c -> two g m c", two=2)
                bv = bag[:].rearrange("p (j c) -> p j c", c=C)
                nc.sync.dma_start(dst[0][:, m0:m0 + 4, :], bv[0:64])
                nc.sync.dma_start(dst[1][:, m0:m0 + 4, :], bv[64:128])

        # software pipeline: first 4 gathers up front (praw bufs), then all
        # gather-independent conv1 work to keep the PE dense while gathers
        # stream, then conv2 tails interleaved with the remaining gathers.
        raws = {}
        for b in range(4):
            raws[b] = gather_part(b)
        fence = None
        for b in range(BPC):
            fence = conv1_part(b)
        for b in range(BPC):
            batch_tail(b, raws.pop(b), fence)
            if b + 4 < BPC:
                raws[b + 4] = gather_part(b + 4)

    hw_specs.TRN2Spec.SWDGE_NS_PER_DESCRIPTOR = _old_ns
    nc.compile()
    return nc


def get_nc():
    if "nc" not in _CACHE:
        _CACHE["nc"] = _build_nc()
    return _CACHE["nc"]


def prep_shared(v1, g1, b1, W2, b2):
    """Host-side weight packing (tiny tensors)."""
    v1 = np.asarray(v1, np.float32)
    vnorm = np.sqrt((v1.astype(np.float64) ** 2).sum(axis=(1, 2), keepdims=True))
    w1 = (np.asarray(g1, np.float32)[:, None, None] * v1 / vnorm).astype(np.float32)
    w1pack = np.ascontiguousarray(w1.transpose(1, 2, 0).reshape(C, 3 * C))

    Wt = np.asarray(W2, np.float32).transpose(1, 0, 2, 3)  # [gi, go, a, tap]
    w2pack = np.zeros((128, 768), np.float32)
    for tb in range(3):
        # block A: rhs slot j -> (E[j], O[j-1])
        w2pack[0:64, tb * 128:tb * 128 + 64] = Wt[:, :, 1, tb]        # E->even a=1
        w2pack[0:64, tb * 128 + 64:tb * 128 + 128] = Wt[:, :, 0, tb]  # E->odd  a=0
        w2pack[64:128, tb * 128:tb * 128 + 64] = Wt[:, :, 0, tb]      # O->even a=0
        # block B: rhs slot j+1 -> (E[j+1], O[j])
        w2pack[0:64, 384 + tb * 128 + 64:384 + tb * 128 + 128] = Wt[:, :, 2, tb]   # E->odd a=2
        w2pack[64:128, 384 + tb * 128:384 + tb * 128 + 64] = Wt[:, :, 2, tb]       # O->even a=2
        w2pack[64:128, 384 + tb * 128 + 64:384 + tb * 128 + 128] = Wt[:, :, 1, tb] # O->odd a=1

    b1col = np.asarray(b1, np.float32).reshape(C, 1)
    b2col = np.concatenate([np.asarray(b2, np.float32)] * 2).reshape(128, 1)
    ident = np.eye(128, dtype=np.float32)
    return w1pack, w2pack, b1col, b2col, ident


def prep_idx(idx_local):
    """[BPC, G, K] int -> slot-aligned int16 index list: position
    j*128 + g = index of ctx row 2j (E[j]); position j*128 + 64 + g =
    index of ctx row 2j-1 (O[j-1]); boundary halves use index 0 (their
    gathered garbage is zeroed on-chip)."""
    a = np.asarray(idx_local)
    m = np.full((BPC, NSL, 128), S, np.int16)  # boundary -> zero row x[b, S]
    m[:, 0:NSLOT, 0:64] = a[:, :, 0::2].transpose(0, 2, 1)
    m[:, 1:NSL, 64:128] = a[:, :, 1::2].transpose(0, 2, 1)
    flat = m.reshape(BPC, IDXN)
    wrapped = flat.reshape(BPC, IDXN // 16, 16).transpose(0, 2, 1)
    return np.ascontiguousarray(np.tile(wrapped, (1, 8, 1)))


def make_in_maps(x, index_all, v1, g1, b1, W2, b2):
    x = np.asarray(x, np.float32)
    w1pack, w2pack, b1col, b2col, ident = prep_shared(v1, g1, b1, W2, b2)
    in_maps = []
    for c in range(NCORES):
        sl = slice(c * BPC, (c + 1) * BPC)
        xp = np.concatenate(
            [x[sl], np.zeros((BPC, 1, C), np.float32)], axis=1)
        in_maps.append({
            "x": np.ascontiguousarray(xp),
            "idx16": prep_idx(np.asarray(index_all)[sl]),
            "w1pack": w1pack,
            "w2pack": w2pack,
            "b1col": b1col,
            "b2col": b2col,
            "ident": ident,
        })
    return in_maps


# test harness hooks: set TRACE=True before calling kernel() to profile.
TRACE = False
LAST_RESULTS = {}


def kernel(x, index_all, v1, g1, b1, W2, b2):
    from concourse.bass_utils import run_bass_kernel_spmd

    x = np.asarray(x, np.float32)
    nc = get_nc()
    in_maps = make_in_maps(x, index_all, v1, g1, b1, W2, b2)
    res = run_bass_kernel_spmd(nc, in_maps, core_ids=list(range(NCORES)),
                               trace=TRACE)
    LAST_RESULTS["res"] = res
    xbag = np.concatenate([res.results[c]["xbag"] for c in range(NCORES)], axis=0)
    xsite = np.concatenate([res.results[c]["xsite"] for c in range(NCORES)], axis=0)
    asite = np.ascontiguousarray(x[:, 20:21, :])
    return xbag, asite, xsite


# revision 30
# speedup vs baseline: 1.3122x; 1.1515x over previous
"""Trainium2 Bass kernel for nn_FeatureConverge_82145544503995.

Data-parallel over the batch dim: 8 batches per NeuronCore x 8 cores.
Per batch, on-device:
  - conv1 (weight-normed 3-tap 1D conv over seq) + tanh(tanh(.)) -> x_site
  - hardware dma_gather of 8320 rows (512 B each) of x[b], index order
    chosen so the gather lands directly in a slot-aligned parity layout:
    partition g holds E[m]=ctx[g,2m,:] at slot m, partition 64+g holds
    O[m]=ctx[g,2m+1,:] at slot m+1.  Gathers round-robin over the 4 SWDGE
    queues so descriptor generation runs on all four Q7 core pairs.
  - conv2 (3x3 grouped conv over the gathered [G,K,C] block) as 6 dense
    K=128 accumulating PE matmuls per 512-element output chunk (weight
    blocks A/B per c-tap), plus 8 N=64 matmuls per batch that recompute
    the two c-edge output columns exactly (the main pass reads one
    garbage element across slot boundaries there).
All matmuls run in float32r (full PE rate at N>=256).
"""
import numpy as np

B, S, C, G, K = 64, 2048, 128, 64, 128
NCORES = 8
BPC = B // NCORES        # batches per core
NSLOT = K // 2           # 64 (even,odd) k-pair data slots
NSL = NSLOT + 1          # 65 slots incl. boundary halves
IDXN = NSL * 128         # 8320 gather indices per batch
_CACHE = {}


def _build_nc():
    from contextlib import ExitStack

    import concourse.bacc as bacc
    import concourse.mybir as mybir
    import concourse.tile as tile
    from concourse.tile_rust import add_dep_helper

    f32 = mybir.dt.float32
    f32r = mybir.dt.float32r
    i16 = mybir.dt.int16
    TANH = mybir.ActivationFunctionType.Tanh
    IDENT = mybir.ActivationFunctionType.Identity

    import concourse.hw_specs as hw_specs
    _old_ns = hw_specs.TRN2Spec.SWDGE_NS_PER_DESCRIPTOR
    if int(__import__('os').environ.get('COSTPATCH', '1')):
        hw_specs.TRN2Spec.SWDGE_NS_PER_DESCRIPTOR = 7.8  # measured Q7 gen rate
    nc = bacc.Bacc("TRN2", target_bir_lowering=False, debug=False,
                   enable_asserts=True, num_devices=NCORES,
                   num_swdge_queues=4)
    x_d = nc.dram_tensor("x", [BPC, S + 1, C], f32, kind="ExternalInput")
    idx_d = nc.dram_tensor("idx16", [BPC, 128, IDXN // 16], i16,
                           kind="ExternalInput")
    w1_d = nc.dram_tensor("w1pack", [C, 3 * C], f32, kind="ExternalInput")
    w2_d = nc.dram_tensor("w2pack", [128, 768], f32, kind="ExternalInput")
    b1_d = nc.dram_tensor("b1col", [C, 1], f32, kind="ExternalInput")
    b2_d = nc.dram_tensor("b2col", [128, 1], f32, kind="ExternalInput")
    id_d = nc.dram_tensor("ident", [128, 128], f32, kind="ExternalInput")
    xbag_d = nc.dram_tensor("xbag", [BPC, G, K, C], f32, kind="ExternalOutput")
    xsite_d = nc.dram_tensor("xsite", [BPC, C, S], f32, kind="ExternalOutput")

    with tile.TileContext(nc) as tc, ExitStack() as es:
        const = es.enter_context(tc.tile_pool(name="const", bufs=1))
        pidx = es.enter_context(tc.tile_pool(name="pidx", bufs=8))
        praw = es.enter_context(tc.tile_pool(name="praw", bufs=4))
        pxs = es.enter_context(tc.tile_pool(name="pxs", bufs=2))
        pxp = es.enter_context(tc.tile_pool(name="pxp", bufs=2))
        psite = es.enter_context(tc.tile_pool(name="psite", bufs=3))
        pbag = es.enter_context(tc.tile_pool(name="pbag", bufs=4))
        psT = es.enter_context(tc.tile_pool(name="psT", bufs=2, space="PSUM"))
        psY = es.enter_context(tc.tile_pool(name="psY", bufs=2, space="PSUM"))
        psA = es.enter_context(tc.tile_pool(name="psA", bufs=3, space="PSUM"))
        psE = es.enter_context(tc.tile_pool(name="psE", bufs=1, space="PSUM"))

        w1s_t = const.tile([C, 3 * C], f32)
        nc.sync.dma_start(w1s_t[:], w1_d.ap())
        w2s_t = const.tile([128, 768], f32)
        nc.sync.dma_start(w2s_t[:], w2_d.ap())
        w1_t = const.tile([C, 3 * C], f32r)
        nc.vector.tensor_copy(w1_t[:], w1s_t[:])
        w2_t = const.tile([128, 768], f32r)
        nc.vector.tensor_copy(w2_t[:], w2s_t[:])
        id_t = const.tile([128, 128], f32r)
        nc.sync.dma_start(id_t[:], id_d.ap().bitcast(f32r))
        b1_t = const.tile([C, 1], f32)
        nc.sync.dma_start(b1_t[:], b1_d.ap())
        b2_t = const.tile([128, 1], f32)
        nc.sync.dma_start(b2_t[:], b2_d.ap())
        z_t = const.tile([128, C], f32)
        nc.vector.memset(z_t[:], 0.0)

        def wA(tb):
            return w2_t[:, tb * 128:(tb + 1) * 128]

        def wB(tb):
            return w2_t[:, 384 + tb * 128:384 + (tb + 1) * 128]

        def edge_rhs(raw_t, off):
            # [128, 64(,1)]: one column per data slot, stride 128 elems
            v = raw_t[:, 2:2 + IDXN].rearrange("p (j c) -> p j c", c=C)
            return v[:, off // C:off // C + NSLOT, off % C:off % C + 1]

        # PE warmup: ~5us of dense matmuls so HAM reaches 8/8 before the
        # first real conv work lands.
        if int(__import__('os').environ.get('WARMUP', '1')):
            pw = psY.tile([128, 512], f32, tag="py")
            for i in range(24):
                nc.tensor.matmul(pw[:], w2_t[:, 0:128],
                                 w2_t[:, 128:640],
                                 start=(i == 0), stop=(i == 23))

        def gather_part(b):
            # ---- slot-aligned parity gather ----
            idx_t = pidx.tile([128, IDXN // 16], i16)
            nc.sync.dma_start(idx_t[:], idx_d.ap()[b])
            raw_t = praw.tile([128, IDXN + 4], f32r)
            # pre-trigger zeroing of the 2-element prefix/suffix (the
            # boundary half-slots gather the real zero row x[b, S, :]).
            nc.vector.tensor_copy(raw_t[:, 0:2], z_t[:, 0:2])
            nc.vector.tensor_copy(raw_t[:, 2 + IDXN:4 + IDXN], z_t[:, 0:2])
            # two half-gathers on different SWDGE queues: both Q7 core
            # pairs generate descriptors concurrently (halves gen latency)
            SPL = 33 * 128
            for h, (lo, hi) in enumerate([(0, SPL), (SPL, IDXN)]):
                nc.gpsimd.dma_gather(
                    out_ap=raw_t[:, 2 + lo:2 + hi].rearrange(
                        "p (j c) -> p j c", c=C),
                    in_ap=x_d.ap()[b].bitcast(f32r),
                    idxs_ap=idx_t[:, lo // 16:hi // 16],
                    num_idxs=hi - lo,
                    num_idxs_reg=hi - lo,
                    elem_size=C,
                    single_packet=False,
                    queue_num=(2 * b + h) % 4,
                )
            return raw_t

        def conv1_part(b):
            # ---- conv1: PE-transpose x[b] -> [C, S+2], 3-tap conv, 2x tanh ----
            xs_t = pxs.tile([128, S], f32r)
            nc.sync.dma_start(
                xs_t[:].rearrange("p (t c) -> p t c", c=C),
                x_d.ap()[b][0:S].rearrange("(t p) c -> p t c", p=128)
                    .bitcast(f32r),
            )
            xp_t = pxp.tile([128, S + 2], f32r)
            nc.vector.tensor_copy(xp_t[:, 0:1], z_t[:, 0:1])
            nc.vector.tensor_copy(xp_t[:, S + 1:S + 2], z_t[:, 0:1])
            for q in range(S // 512):
                pt = psT.tile([128, 512], f32r)
                for u in range(4):
                    nc.tensor.matmul(
                        pt[:, u * 128:(u + 1) * 128],
                        xs_t[:, (q * 4 + u) * 128:(q * 4 + u + 1) * 128],
                        id_t[:], is_transpose=True,
                        start=(u == 0), stop=(u == 3),
                    )
                dst = xp_t[:, 1 + q * 512:1 + (q + 1) * 512]
                nc.scalar.copy(dst, pt[:])
            for n0 in range(0, S, 512):
                py = psY.tile([128, 512], f32)
                for t in range(3):
                    last_mm = nc.tensor.matmul(
                        py[:],
                        w1_t[:, t * C:(t + 1) * C],
                        xp_t[:, n0 + t:n0 + t + 512],
                        start=(t == 0), stop=(t == 2),
                    )
                s1 = psite.tile([128, 512], f32)
                nc.scalar.activation(s1[:], py[:], TANH, bias=b1_t[:])
                s2 = psite.tile([128, 512], f32)
                nc.scalar.activation(s2[:], s1[:], TANH)
                nc.sync.dma_start(xsite_d.ap()[b][:, n0:n0 + 512], s2[:])
            return last_mm


        def batch_tail(b, raw_t, conv1_fence):
            # ---- conv2 edge columns (c=0 -> pe[:,0:64], c=127 -> pe[:,64:128]) ----
            pe = psE.tile([128, 128], f32)
            edge_seq = [
                (0, 0, 1, 0), (0, 0, 2, 1), (0, 1, 1, 128), (0, 1, 2, 129),
                (1, 0, 0, 126), (1, 0, 1, 127), (1, 1, 0, 254), (1, 1, 1, 255),
            ]
            for i, (eo, blk, tb, off) in enumerate(edge_seq):
                emm = nc.tensor.matmul(
                    pe[:, eo * 64:eo * 64 + 64],
                    wA(tb) if blk == 0 else wB(tb),
                    edge_rhs(raw_t, off),
                    start=(i == 0), stop=(i == 7),
                )
                if i == 0 and conv1_fence is not None:
                    # ordering-only edge: keep every conv2 tail behind the
                    # gather-independent conv1 work on the PE stream
                    add_dep_helper(emm.ins, conv1_fence.ins, sync=False,
                                   reason="conv1 fills gather latency")

            # ---- conv2 main: 16 chunk-pairs of 8 k-rows x 128 c ----
            for ci, m0 in enumerate(range(0, NSLOT, 4)):
                ps = psA.tile([128, 512], f32)
                seq = [(0, 1), (0, 0), (0, 2), (1, 1), (1, 0), (1, 2)]
                for i, (blk, tb) in enumerate(seq):
                    base = 2 + (m0 + blk) * 128 + tb - 1
                    cmm = nc.tensor.matmul(
                        ps[:],
                        wA(tb) if blk == 0 else wB(tb),
                        raw_t[:, base:base + 512],
                        start=(i == 0), stop=(i == 5),
                    )
                    if i == 0 and conv1_fence is not None:
                        add_dep_helper(cmm.ins, conv1_fence.ins, sync=False,
                                       reason="conv1 fills gather latency")
                bag = pbag.tile([128, 512], f32)
                bagv = bag[:].rearrange("p (j c) -> p j c", c=C)
                pev = pe[:].rearrange("p (e m) -> p e m", m=64)
                e0 = pev[:, 0, m0:m0 + 4].rearrange("p (m one) -> p m one", one=1)
                e1 = pev[:, 1, m0:m0 + 4].rearrange("p (m one) -> p m one", one=1)
                if ci % 2 == 0:
                    nc.vector.tensor_scalar_add(bag[:], ps[:], b2_t[:])
                    nc.scalar.activation(bagv[:, :, 0:1], e0, IDENT, bias=b2_t[:])
                    nc.vector.tensor_scalar_add(bagv[:, :, C - 1:C], e1, b2_t[:])
                else:
                    nc.scalar.activation(bag[:], ps[:], IDENT, bias=b2_t[:])
                    nc.vector.tensor_scalar_add(bagv[:, :, 0:1], e0, b2_t[:])
                    nc.scalar.activation(bagv[:, :, C - 1:C], e1, IDENT, bias=b2_t[:])
                dst = xbag_d.ap()[b].rearrange("g (m two) c -> two g m c", two=2)
                bv = bag[:].rearrange("p (j c) -> p j c", c=C)
                nc.sync.dma_start(dst[0][:, m0:m0 + 4, :], bv[0:64])
                nc.sync.dma_start(dst[1][:, m0:m0 + 4, :], bv[64:128])

        # software pipeline: first 4 gathers up front (praw bufs), then all
        # gather-independent conv1 work to keep the PE dense while gathers
        # stream, then conv2 tails interleaved with the remaining gathers.
        raws = {}
        for b in range(4):
            raws[b] = gather_part(b)
        fence = None
        for b in range(BPC):
            fence = conv1_part(b)
        for b in range(BPC):
            batch_tail(b, raws.pop(b), fence)
            if b + 4 < BPC:
                raws[b + 4] = gather_part(b + 4)

    hw_specs.TRN2Spec.SWDGE_NS_PER_DESCRIPTOR = _old_ns
    nc.compile()
    return nc


def get_nc():
    if "nc" not in _CACHE:
        _CACHE["nc"] = _build_nc()
    return _CACHE["nc"]


def prep_shared(v1, g1, b1, W2, b2):
    """Host-side weight packing (tiny tensors)."""
    v1 = np.asarray(v1, np.float32)
    vnorm = np.sqrt((v1.astype(np.float64) ** 2).sum(axis=(1, 2), keepdims=True))
    w1 = (np.asarray(g1, np.float32)[:, None, None] * v1 / vnorm).astype(np.float32)
    w1pack = np.ascontiguousarray(w1.transpose(1, 2, 0).reshape(C, 3 * C))

    Wt = np.asarray(W2, np.float32).transpose(1, 0, 2, 3)  # [gi, go, a, tap]
    w2pack = np.zeros((128, 768), np.float32)
    for tb in range(3):
        # block A: rhs slot j -> (E[j], O[j-1])
        w2pack[0:64, tb * 128:tb * 128 + 64] = Wt[:, :, 1, tb]        # E->even a=1
        w2pack[0:64, tb * 128 + 64:tb * 128 + 128] = Wt[:, :, 0, tb]  # E->odd  a=0
        w2pack[64:128, tb * 128:tb * 128 + 64] = Wt[:, :, 0, tb]      # O->even a=0
        # block B: rhs slot j+1 -> (E[j+1], O[j])
        w2pack[0:64, 384 + tb * 128 + 64:384 + tb * 128 + 128] = Wt[:, :, 2, tb]   # E->odd a=2
        w2pack[64:128, 384 + tb * 128:384 + tb * 128 + 64] = Wt[:, :, 2, tb]       # O->even a=2
        w2pack[64:128, 384 + tb * 128 + 64:384 + tb * 128 + 128] = Wt[:, :, 1, tb] # O->odd a=1

    b1col = np.asarray(b1, np.float32).reshape(C, 1)
    b2col = np.concatenate([np.asarray(b2, np.float32)] * 2).reshape(128, 1)
    ident = np.eye(128, dtype=np.float32)
    return w1pack, w2pack, b1col, b2col, ident


def prep_idx(idx_local):
    """[BPC, G, K] int -> slot-aligned int16 index list: position
    j*128 + g = index of ctx row 2j (E[j]); position j*128 + 64 + g =
    index of ctx row 2j-1 (O[j-1]); boundary halves use index 0 (their
    gathered garbage is zeroed on-chip)."""
    a = np.asarray(idx_local)
    m = np.full((BPC, NSL, 128), S, np.int16)  # boundary -> zero row x[b, S]
    m[:, 0:NSLOT, 0:64] = a[:, :, 0::2].transpose(0, 2, 1)
    m[:, 1:NSL, 64:128] = a[:, :, 1::2].transpose(0, 2, 1)
    flat = m.reshape(BPC, IDXN)
    wrapped = flat.reshape(BPC, IDXN // 16, 16).transpose(0, 2, 1)
    return np.ascontiguousarray(np.tile(wrapped, (1, 8, 1)))


def make_in_maps(x, index_all, v1, g1, b1, W2, b2):
    x = np.asarray(x, np.float32)
    w1pack, w2pack, b1col, b2col, ident = prep_shared(v1, g1, b1, W2, b2)
    in_maps = []
    for c in range(NCORES):
        sl = slice(c * BPC, (c + 1) * BPC)
        xp = np.concatenate(
            [x[sl], np.zeros((BPC, 1, C), np.float32)], axis=1)
        in_maps.append({
            "x": np.ascontiguousarray(xp),
            "idx16": prep_idx(np.asarray(index_all)[sl]),
            "w1pack": w1pack,
            "w2pack": w2pack,
            "b1col": b1col,
            "b2col": b2col,
            "ident": ident,
        })
    return in_maps


# test harness hooks: set TRACE=True before calling kernel() to profile.
TRACE = False
LAST_RESULTS = {}


def kernel(x, index_all, v1, g1, b1, W2, b2):
    from concourse.bass_utils import run_bass_kernel_spmd

    x = np.asarray(x, np.float32)
    nc = get_nc()
    in_maps = make_in_maps(x, index_all, v1, g1, b1, W2, b2)
    res = run_bass_kernel_spmd(nc, in_maps, core_ids=list(range(NCORES)),
                               trace=TRACE)
    LAST_RESULTS["res"] = res
    xbag = np.concatenate([res.results[c]["xbag"] for c in range(NCORES)], axis=0)
    xsite = np.concatenate([res.results[c]["xsite"] for c in range(NCORES)], axis=0)
    asite = np.ascontiguousarray(x[:, 20:21, :])
    return xbag, asite, xsite


# revision 31
# speedup vs baseline: 1.4450x; 1.1012x over previous
"""Trainium2 Bass kernel for nn_FeatureConverge_82145544503995.

Data-parallel over the batch dim: 8 batches per NeuronCore x 8 cores.
Per batch, on-device:
  - conv1 (weight-normed 3-tap 1D conv over seq) + tanh(tanh(.)) -> x_site
  - hardware dma_gather of 8320 rows (512 B each) of x[b], index order
    chosen so the gather lands directly in a slot-aligned parity layout:
    partition g holds E[m]=ctx[g,2m,:] at slot m, partition 64+g holds
    O[m]=ctx[g,2m+1,:] at slot m+1.  Gathers round-robin over the 4 SWDGE
    queues so descriptor generation runs on all four Q7 core pairs.
  - conv2 (3x3 grouped conv over the gathered [G,K,C] block) as 6 dense
    K=128 accumulating PE matmuls per 512-element output chunk (weight
    blocks A/B per c-tap), plus 8 N=64 matmuls per batch that recompute
    the two c-edge output columns exactly (the main pass reads one
    garbage element across slot boundaries there).
All matmuls run in float32r (full PE rate at N>=256).
"""
import numpy as np

B, S, C, G, K = 64, 2048, 128, 64, 128
NCORES = 8
BPC = B // NCORES        # batches per core
NSLOT = K // 2           # 64 (even,odd) k-pair data slots
NSL = NSLOT + 1          # 65 slots incl. boundary halves
IDXN = NSL * 128         # 8320 gather indices per batch
_CACHE = {}


def _build_nc():
    from contextlib import ExitStack

    import concourse.bacc as bacc
    import concourse.mybir as mybir
    import concourse.tile as tile
    from concourse.tile_rust import add_dep_helper

    f32 = mybir.dt.float32
    f32r = mybir.dt.float32r
    i16 = mybir.dt.int16
    TANH = mybir.ActivationFunctionType.Tanh
    IDENT = mybir.ActivationFunctionType.Identity

    import concourse.hw_specs as hw_specs
    _old_ns = hw_specs.TRN2Spec.SWDGE_NS_PER_DESCRIPTOR
    if int(__import__('os').environ.get('COSTPATCH', '1')):
        hw_specs.TRN2Spec.SWDGE_NS_PER_DESCRIPTOR = 7.8  # measured Q7 gen rate
    nc = bacc.Bacc("TRN2", target_bir_lowering=False, debug=False,
                   enable_asserts=True, num_devices=NCORES,
                   num_swdge_queues=4)
    x_d = nc.dram_tensor("x", [BPC, S + 1, C], f32, kind="ExternalInput")
    idx_d = nc.dram_tensor("idx16", [BPC, 128, IDXN // 16], i16,
                           kind="ExternalInput")
    w1_d = nc.dram_tensor("w1pack", [C, 3 * C], f32, kind="ExternalInput")
    w2_d = nc.dram_tensor("w2pack", [128, 768], f32, kind="ExternalInput")
    b1_d = nc.dram_tensor("b1col", [C, 1], f32, kind="ExternalInput")
    b2_d = nc.dram_tensor("b2col", [128, 1], f32, kind="ExternalInput")
    id_d = nc.dram_tensor("ident", [128, 128], f32, kind="ExternalInput")
    xbag_d = nc.dram_tensor("xbag", [BPC, G, K, C], f32, kind="ExternalOutput")
    xsite_d = nc.dram_tensor("xsite", [BPC, C, S], f32, kind="ExternalOutput")

    with tile.TileContext(nc) as tc, ExitStack() as es:
        const = es.enter_context(tc.tile_pool(name="const", bufs=1))
        pidx = es.enter_context(tc.tile_pool(name="pidx", bufs=8))
        praw = es.enter_context(tc.tile_pool(name="praw", bufs=4))
        pxs = es.enter_context(tc.tile_pool(name="pxs", bufs=2))
        pxp = es.enter_context(tc.tile_pool(name="pxp", bufs=2))
        psite = es.enter_context(tc.tile_pool(name="psite", bufs=3))
        pbag = es.enter_context(tc.tile_pool(name="pbag", bufs=4))
        psT = es.enter_context(tc.tile_pool(name="psT", bufs=2, space="PSUM"))
        psY = es.enter_context(tc.tile_pool(name="psY", bufs=2, space="PSUM"))
        psA = es.enter_context(tc.tile_pool(name="psA", bufs=3, space="PSUM"))
        psE = es.enter_context(tc.tile_pool(name="psE", bufs=1, space="PSUM"))

        w1s_t = const.tile([C, 3 * C], f32)
        nc.sync.dma_start(w1s_t[:], w1_d.ap())
        w2s_t = const.tile([128, 768], f32)
        nc.sync.dma_start(w2s_t[:], w2_d.ap())
        w1_t = const.tile([C, 3 * C], f32r)
        nc.vector.tensor_copy(w1_t[:], w1s_t[:])
        w2_t = const.tile([128, 768], f32r)
        nc.vector.tensor_copy(w2_t[:], w2s_t[:])
        id_t = const.tile([128, 128], f32r)
        nc.sync.dma_start(id_t[:], id_d.ap().bitcast(f32r))
        b1_t = const.tile([C, 1], f32)
        nc.sync.dma_start(b1_t[:], b1_d.ap())
        b2_t = const.tile([128, 1], f32)
        nc.sync.dma_start(b2_t[:], b2_d.ap())
        z_t = const.tile([128, C], f32)
        nc.vector.memset(z_t[:], 0.0)

        def wA(tb):
            return w2_t[:, tb * 128:(tb + 1) * 128]

        def wB(tb):
            return w2_t[:, 384 + tb * 128:384 + (tb + 1) * 128]

        def edge_rhs(raw_t, off):
            # [128, 64(,1)]: one column per data slot, stride 128 elems
            v = raw_t[:, 2:2 + IDXN].rearrange("p (j c) -> p j c", c=C)
            return v[:, off // C:off // C + NSLOT, off % C:off % C + 1]

        # PE warmup: ~5us of dense matmuls so HAM reaches 8/8 before the
        # first real conv work lands.
        if int(__import__('os').environ.get('WARMUP', '1')):
            pw = psY.tile([128, 512], f32, tag="py")
            for i in range(24):
                nc.tensor.matmul(pw[:], w2_t[:, 0:128],
                                 w2_t[:, 128:640],
                                 start=(i == 0), stop=(i == 23))

        def gather_part(b):
            # ---- slot-aligned parity gather ----
            idx_t = pidx.tile([128, IDXN // 16], i16)
            nc.sync.dma_start(idx_t[:], idx_d.ap()[b])
            raw_t = praw.tile([128, IDXN + 4], f32r)
            # pre-trigger zeroing of the 2-element prefix/suffix (the
            # boundary half-slots gather the real zero row x[b, S, :]).
            nc.vector.tensor_copy(raw_t[:, 0:2], z_t[:, 0:2])
            nc.vector.tensor_copy(raw_t[:, 2 + IDXN:4 + IDXN], z_t[:, 0:2])
            # four quarter-gathers, one per SWDGE queue: all four Q7 core
            # pairs generate descriptors concurrently (quarters gen latency)
            cuts = [0, 17 * 128, 33 * 128, 49 * 128, IDXN]
            for h, (lo, hi) in enumerate(zip(cuts, cuts[1:])):
                nc.gpsimd.dma_gather(
                    out_ap=raw_t[:, 2 + lo:2 + hi].rearrange(
                        "p (j c) -> p j c", c=C),
                    in_ap=x_d.ap()[b].bitcast(f32r),
                    idxs_ap=idx_t[:, lo // 16:hi // 16],
                    num_idxs=hi - lo,
                    num_idxs_reg=hi - lo,
                    elem_size=C,
                    single_packet=False,
                    queue_num=h,
                )
            return raw_t

        def conv1_part(b):
            # ---- conv1: PE-transpose x[b] -> [C, S+2], 3-tap conv, 2x tanh ----
            xs_t = pxs.tile([128, S], f32r)
            nc.sync.dma_start(
                xs_t[:].rearrange("p (t c) -> p t c", c=C),
                x_d.ap()[b][0:S].rearrange("(t p) c -> p t c", p=128)
                    .bitcast(f32r),
            )
            xp_t = pxp.tile([128, S + 2], f32r)
            nc.vector.tensor_copy(xp_t[:, 0:1], z_t[:, 0:1])
            nc.vector.tensor_copy(xp_t[:, S + 1:S + 2], z_t[:, 0:1])
            for q in range(S // 512):
                pt = psT.tile([128, 512], f32r)
                for u in range(4):
                    nc.tensor.matmul(
                        pt[:, u * 128:(u + 1) * 128],
                        xs_t[:, (q * 4 + u) * 128:(q * 4 + u + 1) * 128],
                        id_t[:], is_transpose=True,
                        start=(u == 0), stop=(u == 3),
                    )
                dst = xp_t[:, 1 + q * 512:1 + (q + 1) * 512]
                nc.scalar.copy(dst, pt[:])
            for n0 in range(0, S, 512):
                py = psY.tile([128, 512], f32)
                for t in range(3):
                    last_mm = nc.tensor.matmul(
                        py[:],
                        w1_t[:, t * C:(t + 1) * C],
                        xp_t[:, n0 + t:n0 + t + 512],
                        start=(t == 0), stop=(t == 2),
                    )
                s1 = psite.tile([128, 512], f32)
                nc.scalar.activation(s1[:], py[:], TANH, bias=b1_t[:])
                s2 = psite.tile([128, 512], f32)
                nc.scalar.activation(s2[:], s1[:], TANH)
                nc.sync.dma_start(xsite_d.ap()[b][:, n0:n0 + 512], s2[:])
            return last_mm


        def batch_tail(b, raw_t, conv1_fence):
            # ---- conv2 edge columns (c=0 -> pe[:,0:64], c=127 -> pe[:,64:128]) ----
            pe = psE.tile([128, 128], f32)
            edge_seq = [
                (0, 0, 1, 0), (0, 0, 2, 1), (0, 1, 1, 128), (0, 1, 2, 129),
                (1, 0, 0, 126), (1, 0, 1, 127), (1, 1, 0, 254), (1, 1, 1, 255),
            ]
            for i, (eo, blk, tb, off) in enumerate(edge_seq):
                emm = nc.tensor.matmul(
                    pe[:, eo * 64:eo * 64 + 64],
                    wA(tb) if blk == 0 else wB(tb),
                    edge_rhs(raw_t, off),
                    start=(i == 0), stop=(i == 7),
                )
                if i == 0 and conv1_fence is not None:
                    # ordering-only edge: keep every conv2 tail behind the
                    # gather-independent conv1 work on the PE stream
                    add_dep_helper(emm.ins, conv1_fence.ins, sync=False,
                                   reason="conv1 fills gather latency")

            # ---- conv2 main: 16 chunk-pairs of 8 k-rows x 128 c ----
            for ci, m0 in enumerate(range(0, NSLOT, 4)):
                ps = psA.tile([128, 512], f32)
                seq = [(0, 1), (0, 0), (0, 2), (1, 1), (1, 0), (1, 2)]
                for i, (blk, tb) in enumerate(seq):
                    base = 2 + (m0 + blk) * 128 + tb - 1
                    cmm = nc.tensor.matmul(
                        ps[:],
                        wA(tb) if blk == 0 else wB(tb),
                        raw_t[:, base:base + 512],
                        start=(i == 0), stop=(i == 5),
                    )
                    if i == 0 and conv1_fence is not None:
                        add_dep_helper(cmm.ins, conv1_fence.ins, sync=False,
                                       reason="conv1 fills gather latency")
                bag = pbag.tile([128, 512], f32)
                bagv = bag[:].rearrange("p (j c) -> p j c", c=C)
                pev = pe[:].rearrange("p (e m) -> p e m", m=64)
                e0 = pev[:, 0, m0:m0 + 4].rearrange("p (m one) -> p m one", one=1)
                e1 = pev[:, 1, m0:m0 + 4].rearrange("p (m one) -> p m one", one=1)
                if ci % 2 == 0:
                    nc.vector.tensor_scalar_add(bag[:], ps[:], b2_t[:])
                    nc.scalar.activation(bagv[:, :, 0:1], e0, IDENT, bias=b2_t[:])
                    nc.vector.tensor_scalar_add(bagv[:, :, C - 1:C], e1, b2_t[:])
                else:
                    nc.scalar.activation(bag[:], ps[:], IDENT, bias=b2_t[:])
                    nc.vector.tensor_scalar_add(bagv[:, :, 0:1], e0, b2_t[:])
                    nc.scalar.activation(bagv[:, :, C - 1:C], e1, IDENT, bias=b2_t[:])
                dst = xbag_d.ap()[b].rearrange("g (m two) c -> two g m c", two=2)
                bv = bag[:].rearrange("p (j c) -> p j c", c=C)
                nc.sync.dma_start(dst[0][:, m0:m0 + 4, :], bv[0:64])
                nc.sync.dma_start(dst[1][:, m0:m0 + 4, :], bv[64:128])

        # software pipeline: first 4 gathers up front (praw bufs), then all
        # gather-independent conv1 work to keep the PE dense while gathers
        # stream, then conv2 tails interleaved with the remaining gathers.
        raws = {}
        for b in range(4):
            raws[b] = gather_part(b)
        fence = None
        for b in range(BPC):
            fence = conv1_part(b)
        for b in range(BPC):
            batch_tail(b, raws.pop(b), fence)
            if b + 4 < BPC:
                raws[b + 4] = gather_part(b + 4)

    hw_specs.TRN2Spec.SWDGE_NS_PER_DESCRIPTOR = _old_ns
    nc.compile()
    return nc


def get_nc():
    if "nc" not in _CACHE:
        _CACHE["nc"] = _build_nc()
    return _CACHE["nc"]


def prep_shared(v1, g1, b1, W2, b2):
    """Host-side weight packing (tiny tensors)."""
    v1 = np.asarray(v1, np.float32)
    vnorm = np.sqrt((v1.astype(np.float64) ** 2).sum(axis=(1, 2), keepdims=True))
    w1 = (np.asarray(g1, np.float32)[:, None, None] * v1 / vnorm).astype(np.float32)
    w1pack = np.ascontiguousarray(w1.transpose(1, 2, 0).reshape(C, 3 * C))

    Wt = np.asarray(W2, np.float32).transpose(1, 0, 2, 3)  # [gi, go, a, tap]
    w2pack = np.zeros((128, 768), np.float32)
    for tb in range(3):
        # block A: rhs slot j -> (E[j], O[j-1])
        w2pack[0:64, tb * 128:tb * 128 + 64] = Wt[:, :, 1, tb]        # E->even a=1
        w2pack[0:64, tb * 128 + 64:tb * 128 + 128] = Wt[:, :, 0, tb]  # E->odd  a=0
        w2pack[64:128, tb * 128:tb * 128 + 64] = Wt[:, :, 0, tb]      # O->even a=0
        # block B: rhs slot j+1 -> (E[j+1], O[j])
        w2pack[0:64, 384 + tb * 128 + 64:384 + tb * 128 + 128] = Wt[:, :, 2, tb]   # E->odd a=2
        w2pack[64:128, 384 + tb * 128:384 + tb * 128 + 64] = Wt[:, :, 2, tb]       # O->even a=2
        w2pack[64:128, 384 + tb * 128 + 64:384 + tb * 128 + 128] = Wt[:, :, 1, tb] # O->odd a=1

    b1col = np.asarray(b1, np.float32).reshape(C, 1)
    b2col = np.concatenate([np.asarray(b2, np.float32)] * 2).reshape(128, 1)
    ident = np.eye(128, dtype=np.float32)
    return w1pack, w2pack, b1col, b2col, ident


def prep_idx(idx_local):
    """[BPC, G, K] int -> slot-aligned int16 index list: position
    j*128 + g = index of ctx row 2j (E[j]); position j*128 + 64 + g =
    index of ctx row 2j-1 (O[j-1]); boundary halves use index 0 (their
    gathered garbage is zeroed on-chip)."""
    a = np.asarray(idx_local)
    m = np.full((BPC, NSL, 128), S, np.int16)  # boundary -> zero row x[b, S]
    m[:, 0:NSLOT, 0:64] = a[:, :, 0::2].transpose(0, 2, 1)
    m[:, 1:NSL, 64:128] = a[:, :, 1::2].transpose(0, 2, 1)
    flat = m.reshape(BPC, IDXN)
    wrapped = flat.reshape(BPC, IDXN // 16, 16).transpose(0, 2, 1)
    return np.ascontiguousarray(np.tile(wrapped, (1, 8, 1)))


def make_in_maps(x, index_all, v1, g1, b1, W2, b2):
    x = np.asarray(x, np.float32)
    w1pack, w2pack, b1col, b2col, ident = prep_shared(v1, g1, b1, W2, b2)
    in_maps = []
    for c in range(NCORES):
        sl = slice(c * BPC, (c + 1) * BPC)
        xp = np.concatenate(
            [x[sl], np.zeros((BPC, 1, C), np.float32)], axis=1)
        in_maps.append({
            "x": np.ascontiguousarray(xp),
            "idx16": prep_idx(np.asarray(index_all)[sl]),
            "w1pack": w1pack,
            "w2pack": w2pack,
            "b1col": b1col,
            "b2col": b2col,
            "ident": ident,
        })
    return in_maps


# test harness hooks: set TRACE=True before calling kernel() to profile.
TRACE = False
LAST_RESULTS = {}


def kernel(x, index_all, v1, g1, b1, W2, b2):
    from concourse.bass_utils import run_bass_kernel_spmd

    x = np.asarray(x, np.float32)
    nc = get_nc()
    in_maps = make_in_maps(x, index_all, v1, g1, b1, W2, b2)
    res = run_bass_kernel_spmd(nc, in_maps, core_ids=list(range(NCORES)),
                               trace=TRACE)
    LAST_RESULTS["res"] = res
    xbag = np.concatenate([res.results[c]["xbag"] for c in range(NCORES)], axis=0)
    xsite = np.concatenate([res.results[c]["xsite"] for c in range(NCORES)], axis=0)
    asite = np.ascontiguousarray(x[:, 20:21, :])
    return xbag, asite, xsite


# revision 32
# speedup vs baseline: 1.4732x; 1.0195x over previous
"""Trainium2 Bass kernel for nn_FeatureConverge_82145544503995.

Data-parallel over the batch dim: 8 batches per NeuronCore x 8 cores.
Per batch, on-device:
  - conv1 (weight-normed 3-tap 1D conv over seq) + tanh(tanh(.)) -> x_site
  - hardware dma_gather of 8320 rows (512 B each) of x[b], index order
    chosen so the gather lands directly in a slot-aligned parity layout:
    partition g holds E[m]=ctx[g,2m,:] at slot m, partition 64+g holds
    O[m]=ctx[g,2m+1,:] at slot m+1.  Gathers round-robin over the 4 SWDGE
    queues so descriptor generation runs on all four Q7 core pairs.
  - conv2 (3x3 grouped conv over the gathered [G,K,C] block) as 6 dense
    K=128 accumulating PE matmuls per 512-element output chunk (weight
    blocks A/B per c-tap), plus 8 N=64 matmuls per batch that recompute
    the two c-edge output columns exactly (the main pass reads one
    garbage element across slot boundaries there).
All matmuls run in float32r (full PE rate at N>=256).
"""
import numpy as np

B, S, C, G, K = 64, 2048, 128, 64, 128
NCORES = 8
BPC = B // NCORES        # batches per core
NSLOT = K // 2           # 64 (even,odd) k-pair data slots
NSL = NSLOT + 1          # 65 slots incl. boundary halves
IDXN = NSL * 128         # 8320 gather indices per batch
_CACHE = {}


def _build_nc():
    from contextlib import ExitStack

    import concourse.bacc as bacc
    import concourse.mybir as mybir
    import concourse.tile as tile
    from concourse.tile_rust import add_dep_helper

    f32 = mybir.dt.float32
    f32r = mybir.dt.float32r
    i16 = mybir.dt.int16
    TANH = mybir.ActivationFunctionType.Tanh
    IDENT = mybir.ActivationFunctionType.Identity

    import concourse.hw_specs as hw_specs
    _old_ns = hw_specs.TRN2Spec.SWDGE_NS_PER_DESCRIPTOR
    hw_specs.TRN2Spec.SWDGE_NS_PER_DESCRIPTOR = 7.8  # measured Q7 gen rate
    # (restored below; realistic gather cost makes the Tile scheduler
    # interleave batches correctly)
    nc = bacc.Bacc("TRN2", target_bir_lowering=False, debug=False,
                   enable_asserts=True, num_devices=NCORES,
                   num_swdge_queues=4)
    x_d = nc.dram_tensor("x", [BPC, S + 1, C], f32, kind="ExternalInput")
    idx_d = nc.dram_tensor("idx16", [BPC, 128, IDXN // 16], i16,
                           kind="ExternalInput")
    w1_d = nc.dram_tensor("w1pack", [C, 3 * C], f32, kind="ExternalInput")
    w2_d = nc.dram_tensor("w2pack", [128, 768], f32, kind="ExternalInput")
    b1_d = nc.dram_tensor("b1col", [C, 1], f32, kind="ExternalInput")
    b2_d = nc.dram_tensor("b2col", [128, 1], f32, kind="ExternalInput")
    id_d = nc.dram_tensor("ident", [128, 128], f32, kind="ExternalInput")
    xbag_d = nc.dram_tensor("xbag", [BPC, G, K, C], f32, kind="ExternalOutput")
    xsite_d = nc.dram_tensor("xsite", [BPC, C, S], f32, kind="ExternalOutput")

    with tile.TileContext(nc) as tc, ExitStack() as es:
        const = es.enter_context(tc.tile_pool(name="const", bufs=1))
        pidx = es.enter_context(tc.tile_pool(name="pidx", bufs=8))
        praw = es.enter_context(tc.tile_pool(name="praw", bufs=4))
        pxs = es.enter_context(tc.tile_pool(name="pxs", bufs=2))
        pxp = es.enter_context(tc.tile_pool(name="pxp", bufs=2))
        psite = es.enter_context(tc.tile_pool(name="psite", bufs=3))
        pbag = es.enter_context(tc.tile_pool(name="pbag", bufs=4))
        psT = es.enter_context(tc.tile_pool(name="psT", bufs=2, space="PSUM"))
        psY = es.enter_context(tc.tile_pool(name="psY", bufs=2, space="PSUM"))
        psA = es.enter_context(tc.tile_pool(name="psA", bufs=3, space="PSUM"))
        psE = es.enter_context(tc.tile_pool(name="psE", bufs=1, space="PSUM"))

        w1s_t = const.tile([C, 3 * C], f32)
        nc.sync.dma_start(w1s_t[:], w1_d.ap())
        w2s_t = const.tile([128, 768], f32)
        nc.sync.dma_start(w2s_t[:], w2_d.ap())
        w1_t = const.tile([C, 3 * C], f32r)
        nc.vector.tensor_copy(w1_t[:], w1s_t[:])
        w2_t = const.tile([128, 768], f32r)
        nc.vector.tensor_copy(w2_t[:], w2s_t[:])
        id_t = const.tile([128, 128], f32r)
        nc.sync.dma_start(id_t[:], id_d.ap().bitcast(f32r))
        b1_t = const.tile([C, 1], f32)
        nc.sync.dma_start(b1_t[:], b1_d.ap())
        b2_t = const.tile([128, 1], f32)
        nc.sync.dma_start(b2_t[:], b2_d.ap())
        z_t = const.tile([128, C], f32)
        nc.vector.memset(z_t[:], 0.0)

        def wA(tb):
            return w2_t[:, tb * 128:(tb + 1) * 128]

        def wB(tb):
            return w2_t[:, 384 + tb * 128:384 + (tb + 1) * 128]

        def edge_rhs(raw_t, off):
            # [128, 64(,1)]: one column per data slot, stride 128 elems
            v = raw_t[:, 2:2 + IDXN].rearrange("p (j c) -> p j c", c=C)
            return v[:, off // C:off // C + NSLOT, off % C:off % C + 1]

        # PE warmup: ~5us of dense matmuls so HAM reaches 8/8 before the
        # first real conv work lands.
        pw = psY.tile([128, 512], f32, tag="py")
        for i in range(24):
            nc.tensor.matmul(pw[:], w2_t[:, 0:128], w2_t[:, 128:640],
                             start=(i == 0), stop=(i == 23))

        def gather_part(b):
            # ---- slot-aligned parity gather ----
            idx_t = pidx.tile([128, IDXN // 16], i16)
            nc.sync.dma_start(idx_t[:], idx_d.ap()[b])
            raw_t = praw.tile([128, IDXN + 4], f32r)
            # pre-trigger zeroing of the 2-element prefix/suffix (the
            # boundary half-slots gather the real zero row x[b, S, :]).
            nc.vector.tensor_copy(raw_t[:, 0:2], z_t[:, 0:2])
            nc.vector.tensor_copy(raw_t[:, 2 + IDXN:4 + IDXN], z_t[:, 0:2])
            # four quarter-gathers, one per SWDGE queue: all four Q7 core
            # pairs generate descriptors concurrently (quarters gen latency)
            cuts = [0, 17 * 128, 33 * 128, 49 * 128, IDXN]
            for h, (lo, hi) in enumerate(zip(cuts, cuts[1:])):
                nc.gpsimd.dma_gather(
                    out_ap=raw_t[:, 2 + lo:2 + hi].rearrange(
                        "p (j c) -> p j c", c=C),
                    in_ap=x_d.ap()[b].bitcast(f32r),
                    idxs_ap=idx_t[:, lo // 16:hi // 16],
                    num_idxs=hi - lo,
                    num_idxs_reg=hi - lo,
                    elem_size=C,
                    single_packet=False,
                    queue_num=h,
                )
            return raw_t

        def conv1_part(b):
            # ---- conv1: PE-transpose x[b] -> [C, S+2], 3-tap conv, 2x tanh ----
            xs_t = pxs.tile([128, S], f32r)
            nc.sync.dma_start(
                xs_t[:].rearrange("p (t c) -> p t c", c=C),
                x_d.ap()[b][0:S].rearrange("(t p) c -> p t c", p=128)
                    .bitcast(f32r),
            )
            xp_t = pxp.tile([128, S + 2], f32r)
            nc.vector.tensor_copy(xp_t[:, 0:1], z_t[:, 0:1])
            nc.vector.tensor_copy(xp_t[:, S + 1:S + 2], z_t[:, 0:1])
            for q in range(S // 512):
                pt = psT.tile([128, 512], f32r)
                for u in range(4):
                    nc.tensor.matmul(
                        pt[:, u * 128:(u + 1) * 128],
                        xs_t[:, (q * 4 + u) * 128:(q * 4 + u + 1) * 128],
                        id_t[:], is_transpose=True,
                        start=(u == 0), stop=(u == 3),
                    )
                dst = xp_t[:, 1 + q * 512:1 + (q + 1) * 512]
                nc.scalar.copy(dst, pt[:])
            for n0 in range(0, S, 512):
                py = psY.tile([128, 512], f32)
                for t in range(3):
                    last_mm = nc.tensor.matmul(
                        py[:],
                        w1_t[:, t * C:(t + 1) * C],
                        xp_t[:, n0 + t:n0 + t + 512],
                        start=(t == 0), stop=(t == 2),
                    )
                s1 = psite.tile([128, 512], f32)
                nc.scalar.activation(s1[:], py[:], TANH, bias=b1_t[:])
                s2 = psite.tile([128, 512], f32)
                nc.scalar.activation(s2[:], s1[:], TANH)
                nc.sync.dma_start(xsite_d.ap()[b][:, n0:n0 + 512], s2[:])
            return last_mm


        def batch_tail(b, raw_t, conv1_fence):
            # ---- conv2 edge columns (c=0 -> pe[:,0:64], c=127 -> pe[:,64:128]) ----
            pe = psE.tile([128, 128], f32)
            edge_seq = [
                (0, 0, 1, 0), (0, 0, 2, 1), (0, 1, 1, 128), (0, 1, 2, 129),
                (1, 0, 0, 126), (1, 0, 1, 127), (1, 1, 0, 254), (1, 1, 1, 255),
            ]
            for i, (eo, blk, tb, off) in enumerate(edge_seq):
                emm = nc.tensor.matmul(
                    pe[:, eo * 64:eo * 64 + 64],
                    wA(tb) if blk == 0 else wB(tb),
                    edge_rhs(raw_t, off),
                    start=(i == 0), stop=(i == 7),
                )
                if i == 0 and conv1_fence is not None:
                    # ordering-only edge: keep every conv2 tail behind the
                    # gather-independent conv1 work on the PE stream
                    add_dep_helper(emm.ins, conv1_fence.ins, sync=False,
                                   reason="conv1 fills gather latency")

            # ---- conv2 main: 16 chunk-pairs of 8 k-rows x 128 c ----
            for ci, m0 in enumerate(range(0, NSLOT, 4)):
                ps = psA.tile([128, 512], f32)
                seq = [(0, 1), (0, 0), (0, 2), (1, 1), (1, 0), (1, 2)]
                for i, (blk, tb) in enumerate(seq):
                    base = 2 + (m0 + blk) * 128 + tb - 1
                    cmm = nc.tensor.matmul(
                        ps[:],
                        wA(tb) if blk == 0 else wB(tb),
                        raw_t[:, base:base + 512],
                        start=(i == 0), stop=(i == 5),
                    )
                    if i == 0 and conv1_fence is not None:
                        add_dep_helper(cmm.ins, conv1_fence.ins, sync=False,
                                       reason="conv1 fills gather latency")
                bag = pbag.tile([128, 512], f32)
                bagv = bag[:].rearrange("p (j c) -> p j c", c=C)
                pev = pe[:].rearrange("p (e m) -> p e m", m=64)
                e0 = pev[:, 0, m0:m0 + 4].rearrange("p (m one) -> p m one", one=1)
                e1 = pev[:, 1, m0:m0 + 4].rearrange("p (m one) -> p m one", one=1)
                if ci % 2 == 0:
                    nc.vector.tensor_scalar_add(bag[:], ps[:], b2_t[:])
                    nc.scalar.activation(bagv[:, :, 0:1], e0, IDENT, bias=b2_t[:])
                    nc.vector.tensor_scalar_add(bagv[:, :, C - 1:C], e1, b2_t[:])
                else:
                    nc.scalar.activation(bag[:], ps[:], IDENT, bias=b2_t[:])
                    nc.vector.tensor_scalar_add(bagv[:, :, 0:1], e0, b2_t[:])
                    nc.scalar.activation(bagv[:, :, C - 1:C], e1, IDENT, bias=b2_t[:])
                dst = xbag_d.ap()[b].rearrange("g (m two) c -> two g m c", two=2)
                bv = bag[:].rearrange("p (j c) -> p j c", c=C)
                nc.sync.dma_start(dst[0][:, m0:m0 + 4, :], bv[0:64])
                nc.sync.dma_start(dst[1][:, m0:m0 + 4, :], bv[64:128])

        # software pipeline: first 4 gathers up front (praw bufs), then all
        # gather-independent conv1 work to keep the PE dense while gathers
        # stream, then conv2 tails interleaved with the remaining gathers.
        raws = {}
        for b in range(4):
            raws[b] = gather_part(b)
        fence = None
        for b in range(BPC):
            fence = conv1_part(b)
        for b in range(BPC):
            batch_tail(b, raws.pop(b), fence)
            if b + 4 < BPC:
                raws[b + 4] = gather_part(b + 4)

    hw_specs.TRN2Spec.SWDGE_NS_PER_DESCRIPTOR = _old_ns
    nc.compile()
    return nc


def get_nc():
    if "nc" not in _CACHE:
        _CACHE["nc"] = _build_nc()
    return _CACHE["nc"]


def prep_shared(v1, g1, b1, W2, b2):
    """Host-side weight packing (tiny tensors)."""
    v1 = np.asarray(v1, np.float32)
    vnorm = np.sqrt((v1.astype(np.float64) ** 2).sum(axis=(1, 2), keepdims=True))
    w1 = (np.asarray(g1, np.float32)[:, None, None] * v1 / vnorm).astype(np.float32)
    w1pack = np.ascontiguousarray(w1.transpose(1, 2, 0).reshape(C, 3 * C))

    Wt = np.asarray(W2, np.float32).transpose(1, 0, 2, 3)  # [gi, go, a, tap]
    w2pack = np.zeros((128, 768), np.float32)
    for tb in range(3):
        # block A: rhs slot j -> (E[j], O[j-1])
        w2pack[0:64, tb * 128:tb * 128 + 64] = Wt[:, :, 1, tb]        # E->even a=1
        w2pack[0:64, tb * 128 + 64:tb * 128 + 128] = Wt[:, :, 0, tb]  # E->odd  a=0
        w2pack[64:128, tb * 128:tb * 128 + 64] = Wt[:, :, 0, tb]      # O->even a=0
        # block B: rhs slot j+1 -> (E[j+1], O[j])
        w2pack[0:64, 384 + tb * 128 + 64:384 + tb * 128 + 128] = Wt[:, :, 2, tb]   # E->odd a=2
        w2pack[64:128, 384 + tb * 128:384 + tb * 128 + 64] = Wt[:, :, 2, tb]       # O->even a=2
        w2pack[64:128, 384 + tb * 128 + 64:384 + tb * 128 + 128] = Wt[:, :, 1, tb] # O->odd a=1

    b1col = np.asarray(b1, np.float32).reshape(C, 1)
    b2col = np.concatenate([np.asarray(b2, np.float32)] * 2).reshape(128, 1)
    ident = np.eye(128, dtype=np.float32)
    return w1pack, w2pack, b1col, b2col, ident


def prep_idx(idx_local):
    """[BPC, G, K] int -> slot-aligned int16 index list: position
    j*128 + g = index of ctx row 2j (E[j]); position j*128 + 64 + g =
    index of ctx row 2j-1 (O[j-1]); boundary halves use index 0 (their
    gathered garbage is zeroed on-chip)."""
    a = np.asarray(idx_local)
    m = np.full((BPC, NSL, 128), S, np.int16)  # boundary -> zero row x[b, S]
    m[:, 0:NSLOT, 0:64] = a[:, :, 0::2].transpose(0, 2, 1)
    m[:, 1:NSL, 64:128] = a[:, :, 1::2].transpose(0, 2, 1)
    flat = m.reshape(BPC, IDXN)
    wrapped = flat.reshape(BPC, IDXN // 16, 16).transpose(0, 2, 1)
    return np.ascontiguousarray(np.tile(wrapped, (1, 8, 1)))


def make_in_maps(x, index_all, v1, g1, b1, W2, b2):
    x = np.asarray(x, np.float32)
    w1pack, w2pack, b1col, b2col, ident = prep_shared(v1, g1, b1, W2, b2)
    in_maps = []
    for c in range(NCORES):
        sl = slice(c * BPC, (c + 1) * BPC)
        xp = np.concatenate(
            [x[sl], np.zeros((BPC, 1, C), np.float32)], axis=1)
        in_maps.append({
            "x": np.ascontiguousarray(xp),
            "idx16": prep_idx(np.asarray(index_all)[sl]),
            "w1pack": w1pack,
            "w2pack": w2pack,
            "b1col": b1col,
            "b2col": b2col,
            "ident": ident,
        })
    return in_maps


# test harness hooks: set TRACE=True before calling kernel() to profile.
TRACE = False
LAST_RESULTS = {}


def kernel(x, index_all, v1, g1, b1, W2, b2):
    from concourse.bass_utils import run_bass_kernel_spmd

    x = np.asarray(x, np.float32)
    nc = get_nc()
    in_maps = make_in_maps(x, index_all, v1, g1, b1, W2, b2)
    res = run_bass_kernel_spmd(nc, in_maps, core_ids=list(range(NCORES)),
                               trace=TRACE)
    LAST_RESULTS["res"] = res
    xbag = np.concatenate([res.results[c]["xbag"] for c in range(NCORES)], axis=0)
    xsite = np.concatenate([res.results[c]["xsite"] for c in range(NCORES)], axis=0)
    asite = np.ascontiguousarray(x[:, 20:21, :])
    return xbag, asite, xsite


# revision 34
# speedup vs baseline: 1.5711x; 1.0664x over previous
"""Trainium2 Bass kernel for nn_FeatureConverge_82145544503995.

Data-parallel over the batch dim: 8 batches per NeuronCore x 8 cores.
Per batch, on-device:
  - conv1 (weight-normed 3-tap 1D conv over seq) + tanh(tanh(.)) -> x_site
  - hardware dma_gather of 8320 rows (512 B each) of x[b], index order
    chosen so the gather lands directly in a slot-aligned parity layout:
    partition g holds E[m]=ctx[g,2m,:] at slot m, partition 64+g holds
    O[m]=ctx[g,2m+1,:] at slot m+1.  Gathers round-robin over the 4 SWDGE
    queues so descriptor generation runs on all four Q7 core pairs.
  - conv2 (3x3 grouped conv over the gathered [G,K,C] block) as 6 dense
    K=128 accumulating PE matmuls per 512-element output chunk (weight
    blocks A/B per c-tap), plus 8 N=64 matmuls per batch that recompute
    the two c-edge output columns exactly (the main pass reads one
    garbage element across slot boundaries there).
All matmuls run in float32r (full PE rate at N>=256).
"""
import numpy as np

B, S, C, G, K = 64, 2048, 128, 64, 128
NCORES = 8
BPC = B // NCORES        # batches per core
NSLOT = K // 2           # 64 (even,odd) k-pair data slots
NSL = NSLOT + 1          # 65 slots incl. boundary halves
IDXN = NSL * 128         # 8320 gather indices per batch
_CACHE = {}


def _build_nc():
    from contextlib import ExitStack

    import concourse.bacc as bacc
    import concourse.mybir as mybir
    import concourse.tile as tile
    from concourse.tile_rust import add_dep_helper

    f32 = mybir.dt.float32
    f32r = mybir.dt.float32r
    i16 = mybir.dt.int16
    TANH = mybir.ActivationFunctionType.Tanh
    IDENT = mybir.ActivationFunctionType.Identity

    import concourse.hw_specs as hw_specs
    _old_ns = hw_specs.TRN2Spec.SWDGE_NS_PER_DESCRIPTOR
    hw_specs.TRN2Spec.SWDGE_NS_PER_DESCRIPTOR = 7.8  # measured Q7 gen rate
    # (restored below; realistic gather cost makes the Tile scheduler
    # interleave batches correctly)
    nc = bacc.Bacc("TRN2", target_bir_lowering=False, debug=False,
                   enable_asserts=True, num_devices=NCORES,
                   num_swdge_queues=4)
    x_d = nc.dram_tensor("x", [BPC, S + 1, C], f32, kind="ExternalInput")
    idx_d = nc.dram_tensor("idx16", [BPC, 128, IDXN // 16], i16,
                           kind="ExternalInput")
    w1_d = nc.dram_tensor("w1pack", [C, 3 * C], f32, kind="ExternalInput")
    w2_d = nc.dram_tensor("w2pack", [128, 768], f32, kind="ExternalInput")
    b1_d = nc.dram_tensor("b1col", [C, 1], f32, kind="ExternalInput")
    b2_d = nc.dram_tensor("b2col", [128, 1], f32, kind="ExternalInput")
    id_d = nc.dram_tensor("ident", [128, 128], f32, kind="ExternalInput")
    xbag_d = nc.dram_tensor("xbag", [BPC, G, K, C], f32, kind="ExternalOutput")
    xsite_d = nc.dram_tensor("xsite", [BPC, C, S], f32, kind="ExternalOutput")

    with tile.TileContext(nc) as tc, ExitStack() as es:
        const = es.enter_context(tc.tile_pool(name="const", bufs=1))
        pidx = es.enter_context(tc.tile_pool(name="pidx", bufs=8))
        praw = es.enter_context(tc.tile_pool(name="praw", bufs=4))
        pxs = es.enter_context(tc.tile_pool(name="pxs", bufs=2))
        pxp = es.enter_context(tc.tile_pool(name="pxp", bufs=2))
        psite = es.enter_context(tc.tile_pool(name="psite", bufs=4))
        pbag = es.enter_context(tc.tile_pool(name="pbag", bufs=5))
        psT = es.enter_context(tc.tile_pool(name="psT", bufs=2, space="PSUM"))
        psY = es.enter_context(tc.tile_pool(name="psY", bufs=2, space="PSUM"))
        psA = es.enter_context(tc.tile_pool(name="psA", bufs=3, space="PSUM"))
        psE = es.enter_context(tc.tile_pool(name="psE", bufs=1, space="PSUM"))

        w1s_t = const.tile([C, 3 * C], f32)
        nc.sync.dma_start(w1s_t[:], w1_d.ap())
        w2s_t = const.tile([128, 768], f32)
        nc.sync.dma_start(w2s_t[:], w2_d.ap())
        w1_t = const.tile([C, 3 * C], f32r)
        nc.vector.tensor_copy(w1_t[:], w1s_t[:])
        w2_t = const.tile([128, 768], f32r)
        nc.vector.tensor_copy(w2_t[:], w2s_t[:])
        id_t = const.tile([128, 128], f32r)
        nc.sync.dma_start(id_t[:], id_d.ap().bitcast(f32r))
        b1_t = const.tile([C, 1], f32)
        nc.sync.dma_start(b1_t[:], b1_d.ap())
        b2_t = const.tile([128, 1], f32)
        nc.sync.dma_start(b2_t[:], b2_d.ap())
        z_t = const.tile([128, C], f32)
        nc.vector.memset(z_t[:], 0.0)

        def wA(tb):
            return w2_t[:, tb * 128:(tb + 1) * 128]

        def wB(tb):
            return w2_t[:, 384 + tb * 128:384 + (tb + 1) * 128]

        def edge_rhs(raw_t, off):
            # [128, 64(,1)]: one column per data slot, stride 128 elems
            v = raw_t[:, 2:2 + IDXN].rearrange("p (j c) -> p j c", c=C)
            return v[:, off // C:off // C + NSLOT, off % C:off % C + 1]

        # PE warmup: ~5us of dense matmuls so HAM reaches 8/8 before the
        # first real conv work lands.
        pw = psY.tile([128, 512], f32, tag="py")
        for i in range(24):
            nc.tensor.matmul(pw[:], w2_t[:, 0:128], w2_t[:, 128:640],
                             start=(i == 0), stop=(i == 23))

        def gather_part(b):
            # ---- slot-aligned parity gather ----
            idx_t = pidx.tile([128, IDXN // 16], i16)
            nc.sync.dma_start(idx_t[:], idx_d.ap()[b])
            raw_t = praw.tile([128, IDXN + 4], f32r)
            # pre-trigger zeroing of the 2-element prefix/suffix (the
            # boundary half-slots gather the real zero row x[b, S, :]).
            nc.vector.tensor_copy(raw_t[:, 0:2], z_t[:, 0:2])
            nc.vector.tensor_copy(raw_t[:, 2 + IDXN:4 + IDXN], z_t[:, 0:2])
            # four quarter-gathers, one per SWDGE queue: all four Q7 core
            # pairs generate descriptors concurrently (quarters gen latency)
            cuts = [0, 17 * 128, 33 * 128, 49 * 128, IDXN]
            for h, (lo, hi) in enumerate(zip(cuts, cuts[1:])):
                nc.gpsimd.dma_gather(
                    out_ap=raw_t[:, 2 + lo:2 + hi].rearrange(
                        "p (j c) -> p j c", c=C),
                    in_ap=x_d.ap()[b].bitcast(f32r),
                    idxs_ap=idx_t[:, lo // 16:hi // 16],
                    num_idxs=hi - lo,
                    num_idxs_reg=hi - lo,
                    elem_size=C,
                    single_packet=False,
                    queue_num=h,
                )
            return raw_t

        def conv1_part(b):
            # ---- conv1: PE-transpose x[b] -> [C, S+2], 3-tap conv, 2x tanh ----
            xs_t = pxs.tile([128, S], f32r)
            nc.sync.dma_start(
                xs_t[:],
                x_d.ap()[b][0:S].rearrange("(p f) c -> p (f c)", p=128)
                    .bitcast(f32r),
            )
            xp_t = pxp.tile([128, S + 2], f32r)
            nc.vector.tensor_copy(xp_t[:, 0:1], z_t[:, 0:1])
            nc.vector.tensor_copy(xp_t[:, S + 1:S + 2], z_t[:, 0:1])
            for q in range(S // 512):
                pt = psT.tile([128, 512], f32r)
                for u in range(4):
                    nc.tensor.matmul(
                        pt[:, u * 128:(u + 1) * 128],
                        xs_t[:, (q * 4 + u) * 128:(q * 4 + u + 1) * 128],
                        id_t[:], is_transpose=True,
                        start=(u == 0), stop=(u == 3),
                    )
                xpv = xp_t[:, 1:1 + S].rearrange("p (j rr) -> p rr j", rr=16)
                nc.scalar.copy(xpv[:, q * 4:q * 4 + 4, :], pt[:])
            for n0 in range(0, S, 512):
                py = psY.tile([128, 512], f32)
                for t in range(3):
                    last_mm = nc.tensor.matmul(
                        py[:],
                        w1_t[:, t * C:(t + 1) * C],
                        xp_t[:, n0 + t:n0 + t + 512],
                        start=(t == 0), stop=(t == 2),
                    )
                s1 = psite.tile([128, 512], f32)
                nc.scalar.activation(s1[:], py[:], TANH, bias=b1_t[:])
                s2 = psite.tile([128, 512], f32)
                nc.scalar.activation(s2[:], s1[:], TANH)
                nc.sync.dma_start(xsite_d.ap()[b][:, n0:n0 + 512], s2[:])
            return last_mm


        def batch_tail(b, raw_t, conv1_fence):
            # ---- conv2 edge columns (c=0 -> pe[:,0:64], c=127 -> pe[:,64:128]) ----
            pe = psE.tile([128, 128], f32)
            edge_seq = [
                (0, 0, 1, 0), (0, 0, 2, 1), (0, 1, 1, 128), (0, 1, 2, 129),
                (1, 0, 0, 126), (1, 0, 1, 127), (1, 1, 0, 254), (1, 1, 1, 255),
            ]
            for i, (eo, blk, tb, off) in enumerate(edge_seq):
                emm = nc.tensor.matmul(
                    pe[:, eo * 64:eo * 64 + 64],
                    wA(tb) if blk == 0 else wB(tb),
                    edge_rhs(raw_t, off),
                    start=(i == 0), stop=(i == 7),
                )
                if i == 0 and conv1_fence is not None:
                    # ordering-only edge: keep every conv2 tail behind the
                    # gather-independent conv1 work on the PE stream
                    add_dep_helper(emm.ins, conv1_fence.ins, sync=False,
                                   reason="conv1 fills gather latency")

            # ---- conv2 main: 16 chunk-pairs of 8 k-rows x 128 c ----
            for ci, m0 in enumerate(range(0, NSLOT, 4)):
                ps = psA.tile([128, 512], f32)
                seq = [(0, 1), (0, 0), (0, 2), (1, 1), (1, 0), (1, 2)]
                for i, (blk, tb) in enumerate(seq):
                    base = 2 + (m0 + blk) * 128 + tb - 1
                    cmm = nc.tensor.matmul(
                        ps[:],
                        wA(tb) if blk == 0 else wB(tb),
                        raw_t[:, base:base + 512],
                        start=(i == 0), stop=(i == 5),
                    )
                    if i == 0 and conv1_fence is not None:
                        add_dep_helper(cmm.ins, conv1_fence.ins, sync=False,
                                       reason="conv1 fills gather latency")
                bag = pbag.tile([128, 512], f32)
                bagv = bag[:].rearrange("p (j c) -> p j c", c=C)
                pev = pe[:].rearrange("p (e m) -> p e m", m=64)
                e0 = pev[:, 0, m0:m0 + 4].rearrange("p (m one) -> p m one", one=1)
                e1 = pev[:, 1, m0:m0 + 4].rearrange("p (m one) -> p m one", one=1)
                if ci % 2 == 0:
                    nc.vector.tensor_scalar_add(bag[:], ps[:], b2_t[:])
                    nc.scalar.activation(bagv[:, :, 0:1], e0, IDENT, bias=b2_t[:])
                    nc.vector.tensor_scalar_add(bagv[:, :, C - 1:C], e1, b2_t[:])
                else:
                    nc.scalar.activation(bag[:], ps[:], IDENT, bias=b2_t[:])
                    nc.vector.tensor_scalar_add(bagv[:, :, 0:1], e0, b2_t[:])
                    nc.scalar.activation(bagv[:, :, C - 1:C], e1, IDENT, bias=b2_t[:])
                dst = xbag_d.ap()[b].rearrange("g (m two) c -> two g m c", two=2)
                bv = bag[:].rearrange("p (j c) -> p j c", c=C)
                nc.sync.dma_start(dst[0][:, m0:m0 + 4, :], bv[0:64])
                nc.sync.dma_start(dst[1][:, m0:m0 + 4, :], bv[64:128])

        # software pipeline: first 4 gathers up front (praw bufs), then all
        # gather-independent conv1 work to keep the PE dense while gathers
        # stream, then conv2 tails interleaved with the remaining gathers.
        raws = {}
        for b in range(4):
            raws[b] = gather_part(b)
        fence = None
        for b in range(BPC):
            fence = conv1_part(b)
        for b in range(BPC):
            batch_tail(b, raws.pop(b), fence)
            if b + 4 < BPC:
                raws[b + 4] = gather_part(b + 4)

    hw_specs.TRN2Spec.SWDGE_NS_PER_DESCRIPTOR = _old_ns
    nc.compile()
    return nc


def get_nc():
    if "nc" not in _CACHE:
        _CACHE["nc"] = _build_nc()
    return _CACHE["nc"]


def prep_shared(v1, g1, b1, W2, b2):
    """Host-side weight packing (tiny tensors)."""
    v1 = np.asarray(v1, np.float32)
    vnorm = np.sqrt((v1.astype(np.float64) ** 2).sum(axis=(1, 2), keepdims=True))
    w1 = (np.asarray(g1, np.float32)[:, None, None] * v1 / vnorm).astype(np.float32)
    w1pack = np.ascontiguousarray(w1.transpose(1, 2, 0).reshape(C, 3 * C))

    Wt = np.asarray(W2, np.float32).transpose(1, 0, 2, 3)  # [gi, go, a, tap]
    w2pack = np.zeros((128, 768), np.float32)
    for tb in range(3):
        # block A: rhs slot j -> (E[j], O[j-1])
        w2pack[0:64, tb * 128:tb * 128 + 64] = Wt[:, :, 1, tb]        # E->even a=1
        w2pack[0:64, tb * 128 + 64:tb * 128 + 128] = Wt[:, :, 0, tb]  # E->odd  a=0
        w2pack[64:128, tb * 128:tb * 128 + 64] = Wt[:, :, 0, tb]      # O->even a=0
        # block B: rhs slot j+1 -> (E[j+1], O[j])
        w2pack[0:64, 384 + tb * 128 + 64:384 + tb * 128 + 128] = Wt[:, :, 2, tb]   # E->odd a=2
        w2pack[64:128, 384 + tb * 128:384 + tb * 128 + 64] = Wt[:, :, 2, tb]       # O->even a=2
        w2pack[64:128, 384 + tb * 128 + 64:384 + tb * 128 + 128] = Wt[:, :, 1, tb] # O->odd a=1

    b1col = np.asarray(b1, np.float32).reshape(C, 1)
    b2col = np.concatenate([np.asarray(b2, np.float32)] * 2).reshape(128, 1)
    ident = np.eye(128, dtype=np.float32)
    return w1pack, w2pack, b1col, b2col, ident


def prep_idx(idx_local):
    """[BPC, G, K] int -> slot-aligned int16 index list: position
    j*128 + g = index of ctx row 2j (E[j]); position j*128 + 64 + g =
    index of ctx row 2j-1 (O[j-1]); boundary halves use index 0 (their
    gathered garbage is zeroed on-chip)."""
    a = np.asarray(idx_local)
    m = np.full((BPC, NSL, 128), S, np.int16)  # boundary -> zero row x[b, S]
    m[:, 0:NSLOT, 0:64] = a[:, :, 0::2].transpose(0, 2, 1)
    m[:, 1:NSL, 64:128] = a[:, :, 1::2].transpose(0, 2, 1)
    flat = m.reshape(BPC, IDXN)
    wrapped = flat.reshape(BPC, IDXN // 16, 16).transpose(0, 2, 1)
    return np.ascontiguousarray(np.tile(wrapped, (1, 8, 1)))


def make_in_maps(x, index_all, v1, g1, b1, W2, b2):
    x = np.asarray(x, np.float32)
    w1pack, w2pack, b1col, b2col, ident = prep_shared(v1, g1, b1, W2, b2)
    in_maps = []
    for c in range(NCORES):
        sl = slice(c * BPC, (c + 1) * BPC)
        xp = np.concatenate(
            [x[sl], np.zeros((BPC, 1, C), np.float32)], axis=1)
        in_maps.append({
            "x": np.ascontiguousarray(xp),
            "idx16": prep_idx(np.asarray(index_all)[sl]),
            "w1pack": w1pack,
            "w2pack": w2pack,
            "b1col": b1col,
            "b2col": b2col,
            "ident": ident,
        })
    return in_maps


# test harness hooks: set TRACE=True before calling kernel() to profile.
TRACE = False
LAST_RESULTS = {}


def kernel(x, index_all, v1, g1, b1, W2, b2):
    from concourse.bass_utils import run_bass_kernel_spmd

    x = np.asarray(x, np.float32)
    nc = get_nc()
    in_maps = make_in_maps(x, index_all, v1, g1, b1, W2, b2)
    res = run_bass_kernel_spmd(nc, in_maps, core_ids=list(range(NCORES)),
                               trace=TRACE)
    LAST_RESULTS["res"] = res
    xbag = np.concatenate([res.results[c]["xbag"] for c in range(NCORES)], axis=0)
    xsite = np.concatenate([res.results[c]["xsite"] for c in range(NCORES)], axis=0)
    asite = np.ascontiguousarray(x[:, 20:21, :])
    return xbag, asite, xsite
